# revision 1
# baseline (speedup 1.0000x reference)
"""Trainium2 Bass kernel for nn_CrossAttention (linear cross-attention block).

Computation (per batch b):
  xn  = LN(x[b]; norm_g, norm_b)                 [T, D]
  xfn = LN(xf[b]; tnorm_g, tnorm_b)              [N, TD]
  q   = softmax_c((xn @ Wq + bq).reshape(T,H,C))
  k   = softmax_n((xfn @ Wk + bk).reshape(N,H,C))
  v   = (xfn @ Wv + bv).reshape(N,H,C)
  attn= einsum('nhc,nhd->hcd', k, v); y = einsum('thc,hcd->thd', q, attn)
  e   = silu(emb) @ emb_W + emb_b; scale, shift = split(e)
  h   = LN(y; fnorm_g, fnorm_b) * (1+scale) + shift
  out = x + silu(h) @ out_W + out_b

Sharding: pure data-parallel over batch B=32 across 8 NeuronCores (4 each).

Device strategy highlights:
  - LN affine (g) folded into the projection weights on the host; LN
    mean/bias folded into the matmuls as K=1 rank-1 accumulation terms
    (lhsT = -mu row / std row, rhs = colsum/bias rows); the 1/std scale is
    applied by the scalar engine's per-partition `scale` operand fused into
    the Exp/Identity activation that drains PSUM.
  - All matmuls in bf16 (f32 accumulate). Transposed operands produced by
    the DMA xbar transpose engine (2-byte dtype), either straight from DRAM
    (host-precast bf16 x / xf) or SBUF->SBUF in 128x128 blocks.
  - Dual softmax: q-softmax over C is a free-dim grouped reduce + broadcast
    divide in natural [t, hc] layout; k-softmax over N folds into a
    per-partition reciprocal scale of the (exp_k^T v) head matmul outputs.
"""

import os
from contextlib import ExitStack

import numpy as np
import ml_dtypes

import concourse.bass as bass
import concourse.mybir as mybir
import concourse.tile as tile
from concourse import bacc, library_config
from concourse.bass_utils import run_bass_kernel_spmd
from concourse.masks import make_identity

# problem shapes (hardcoded per contract)
B, T, N, D, TD, H, C, TE = 32, 1024, 256, 1024, 768, 16, 64, 2048
D2 = 2 * D
EPS = 1e-5
NCORES = 8
BPC = B // NCORES           # batches per core
TI = T // 128               # 8 t-tiles
KD = D // 128               # 8 k-tiles over D
KTD = TD // 128             # 6 k-tiles over TD
KTE = TE // 128             # 16 k-tiles over TE
NT = N // 128               # 2 n-tiles
NCH = D // 512              # 2 free 512-chunks over D

F32 = mybir.dt.float32
BF16 = mybir.dt.bfloat16
AF = mybir.ActivationFunctionType
ALU = mybir.AluOpType
NBF = ml_dtypes.bfloat16

_PROGRAM = None  # cached (nc) build
K_STAGES = int(os.environ.get("K_STAGES", "99"))  # debug: limit build stages
K_SUB = int(os.environ.get("K_SUB", "99"))  # debug: sub-stage within stage 6


def _build_program():
    nc = bacc.Bacc("TRN2", target_bir_lowering=False, debug=False,
                   num_devices=NCORES)

    # ---- DRAM I/O ----
    d_xbf = nc.dram_tensor("xbf", [BPC, T, D], BF16, kind="ExternalInput")
    d_xf32 = nc.dram_tensor("xf32", [BPC, T, D], F32, kind="ExternalInput")
    d_xfbf = nc.dram_tensor("xfbf", [BPC, N, TD], BF16, kind="ExternalInput")
    d_emb = nc.dram_tensor("emb", [BPC, TE], F32, kind="ExternalInput")
    d_wq = nc.dram_tensor("wq", [D, D], BF16, kind="ExternalInput")
    d_wk = nc.dram_tensor("wk", [TD, D], BF16, kind="ExternalInput")
    d_wv = nc.dram_tensor("wv", [TD, D], BF16, kind="ExternalInput")
    d_wo = nc.dram_tensor("wo", [D, D], BF16, kind="ExternalInput")
    d_wemb = nc.dram_tensor("wemb", [TE, D2], BF16, kind="ExternalInput")
    d_cwq = nc.dram_tensor("cwq", [D], BF16, kind="ExternalInput")
    d_cwk = nc.dram_tensor("cwk", [D], BF16, kind="ExternalInput")
    d_cwv = nc.dram_tensor("cwv", [D], BF16, kind="ExternalInput")
    d_bqe = nc.dram_tensor("bqe", [D], BF16, kind="ExternalInput")
    d_bke = nc.dram_tensor("bke", [D], BF16, kind="ExternalInput")
    d_bve = nc.dram_tensor("bve", [D], BF16, kind="ExternalInput")
    d_outb = nc.dram_tensor("outb", [D], BF16, kind="ExternalInput")
    d_embb = nc.dram_tensor("embb", [D2], BF16, kind="ExternalInput")
    d_fg = nc.dram_tensor("fg", [D], F32, kind="ExternalInput")
    d_fb = nc.dram_tensor("fb", [D], F32, kind="ExternalInput")
    d_out = nc.dram_tensor("out", [BPC, T, D], F32, kind="ExternalOutput")

    with tile.TileContext(nc) as tc, ExitStack() as ctx:
        wpool = ctx.enter_context(tc.tile_pool(name="weights", bufs=1))
        cpool = ctx.enter_context(tc.tile_pool(name="consts", bufs=1))

        # ---- persistent weights ----
        wq_sb = wpool.tile([128, KD, D], BF16)
        nc.sync.dma_start(wq_sb[:], d_wq[:].rearrange("(i p) m -> p i m", p=128))
        wk_sb = wpool.tile([128, KTD, D], BF16)
        nc.sync.dma_start(wk_sb[:], d_wk[:].rearrange("(i p) m -> p i m", p=128))
        wv_sb = wpool.tile([128, KTD, D], BF16)
        nc.sync.dma_start(wv_sb[:], d_wv[:].rearrange("(i p) m -> p i m", p=128))
        wo_sb = wpool.tile([128, KD, D], BF16)
        nc.sync.dma_start(wo_sb[:], d_wo[:].rearrange("(i p) m -> p i m", p=128))

        # ---- constants ----
        def row_const(dram, n, tag):
            t = cpool.tile([1, n], BF16, tag=tag)
            nc.sync.dma_start(t[:], dram[None, :])
            return t
        cwq_r = row_const(d_cwq, D, "cwq_r")
        cwk_r = row_const(d_cwk, D, "cwk_r")
        cwv_r = row_const(d_cwv, D, "cwv_r")
        bqe_r = row_const(d_bqe, D, "bqe_r")
        bke_r = row_const(d_bke, D, "bke_r")
        bve_r = row_const(d_bve, D, "bve_r")
        outb_r = row_const(d_outb, D, "outb_r")
        ones_r = cpool.tile([1, 128], BF16)
        nc.vector.memset(ones_r[:], 1.0)
        ones_c = cpool.tile([128, 1], BF16)
        nc.vector.memset(ones_c[:], 1.0)
        eps_c = cpool.tile([128, 1], F32)
        nc.vector.memset(eps_c[:], EPS)
        ident = cpool.tile([128, 128], BF16)
        make_identity(nc, ident[:])
        nc.gpsimd.load_library(library_config.attnmlp)

        a_bf = cpool.tile([BPC, D], BF16)   # FiLM A rows (one per batch)
        b_bf = cpool.tile([BPC, D], BF16)   # FiLM B rows

        # ---- emb / FiLM phase (all 4 batches at once) ----
        if K_STAGES < 2:
            nc.vector.memset(a_bf[:], 1.0)
            nc.vector.memset(b_bf[:], 0.0)
        if K_STAGES >= 2:
         with tc.tile_pool(name="wemb", bufs=1) as wep, \
             tc.tile_pool(name="etmp", bufs=1) as ep, \
             tc.tile_pool(name="pse", bufs=2, space=bass.MemorySpace.PSUM) as pse:
            wemb_sb = wep.tile([128, KTE, D2], BF16)
            embb_r = ep.tile([1, D2], BF16)
            nc.sync.dma_start(embb_r[:], d_embb[None, :])
            fg_r = ep.tile([1, D], F32)
            nc.sync.dma_start(fg_r[:], d_fg[None, :])
            fb_r = ep.tile([1, D], F32)
            nc.sync.dma_start(fb_r[:], d_fb[None, :])
            nc.sync.dma_start(wemb_sb[:],
                              d_wemb[:].rearrange("(i p) m -> p i m", p=128))
            emb_sb = ep.tile([BPC, TE], F32)
            nc.sync.dma_start(emb_sb[:], d_emb[:])
            semb = ep.tile([BPC, TE], BF16)
            sgm = ep.tile([BPC, TE], F32)
            nc.scalar.activation(sgm[:], emb_sb[:], AF.Sigmoid)
            nc.vector.tensor_mul(semb[:], emb_sb[:], sgm[:])
            embT = ep.tile([128, KTE, BPC], BF16)
            for c in range(KTE):
                pst = pse.tile([128, BPC], BF16, tag="pst")
                nc.tensor.transpose(pst[:], semb[:, c * 128:(c + 1) * 128],
                                    ident[0:BPC, 0:BPC])
                nc.vector.tensor_copy(embT[:, c, :], pst[:])
            e_sb = ep.tile([BPC, D2], F32)
            for ch in range(D2 // 512):
                pe = pse.tile([BPC, 512], F32, tag="pe")
                for kt in range(KTE):
                    nc.tensor.matmul(pe[:], embT[:, kt, :],
                                     wemb_sb[:, kt, ch * 512:(ch + 1) * 512],
                                     start=(kt == 0), stop=False)
                nc.tensor.matmul(pe[:], ones_r[0:1, 0:BPC],
                                 embb_r[0:1, ch * 512:(ch + 1) * 512],
                                 start=False, stop=True)
                nc.vector.tensor_copy(e_sb[:, ch * 512:(ch + 1) * 512], pe[:])
            # FiLM rows: A = fg*(1+scale), Bf = fb*(1+scale) + shift
            fg4 = ep.tile([BPC, D], F32)
            nc.gpsimd.partition_broadcast(fg4[:], fg_r[:], channels=BPC)
            fb4 = ep.tile([BPC, D], F32)
            nc.gpsimd.partition_broadcast(fb4[:], fb_r[:], channels=BPC)
            tall = ep.tile([BPC, D], F32)
            nc.vector.tensor_scalar_add(tall[:], e_sb[:, 0:D], 1.0)
            nc.vector.tensor_mul(a_bf[:], tall[:], fg4[:])
            btmp = ep.tile([BPC, D], F32)
            nc.vector.tensor_mul(btmp[:], tall[:], fb4[:])
            nc.vector.tensor_add(b_bf[:], btmp[:], e_sb[:, D:D2])

        # ---- batch-phase pools ----
        xstat = ctx.enter_context(tc.tile_pool(name="xstat", bufs=2))
        xtp = ctx.enter_context(tc.tile_pool(name="xt", bufs=2))
        colp = ctx.enter_context(tc.tile_pool(name="cols", bufs=2))
        rowp = ctx.enter_context(tc.tile_pool(name="rows", bufs=2))
        kvp = ctx.enter_context(tc.tile_pool(name="kv", bufs=2))
        kvp1 = ctx.enter_context(tc.tile_pool(name="kv1", bufs=1))
        qp = ctx.enter_context(tc.tile_pool(name="q", bufs=2))
        qtp = ctx.enter_context(tc.tile_pool(name="qt", bufs=2))
        hp = ctx.enter_context(tc.tile_pool(name="h", bufs=2))
        htp = ctx.enter_context(tc.tile_pool(name="ht", bufs=1))
        resp = ctx.enter_context(tc.tile_pool(name="res", bufs=2))
        outp = ctx.enter_context(tc.tile_pool(name="o", bufs=2))
        abp = ctx.enter_context(tc.tile_pool(name="ab", bufs=1))
        psq = ctx.enter_context(tc.tile_pool(name="psq", bufs=2, space=bass.MemorySpace.PSUM))
        psy = ctx.enter_context(tc.tile_pool(name="psy", bufs=2, space=bass.MemorySpace.PSUM))
        psa = ctx.enter_context(tc.tile_pool(name="psa", bufs=1, space=bass.MemorySpace.PSUM))
        pso = ctx.enter_context(tc.tile_pool(name="pso", bufs=2, space=bass.MemorySpace.PSUM))

        for b in range(BPC):
            # FiLM broadcast rows for this batch (stage to partition 0 via DMA)
            arow = abp.tile([1, D], BF16, tag="arow")
            nc.sync.dma_start(arow[:], a_bf[b:b + 1, :])
            brow = abp.tile([1, D], BF16, tag="brow")
            nc.sync.dma_start(brow[:], b_bf[b:b + 1, :])
            a_bc = abp.tile([128, D], BF16, tag="abc")
            nc.gpsimd.partition_broadcast(a_bc[:], arow[:], channels=128)
            b_bc = abp.tile([128, D], BF16, tag="bbc")
            nc.gpsimd.partition_broadcast(b_bc[:], brow[:], channels=128)

            # ---------- xf path: stats ----------
            if K_STAGES < 3:
                continue
            mvf = colp.tile([128, NT, 2], F32, tag="mvf")
            for i in range(NT):
                xft = xstat.tile([128, TD], BF16, tag="xfstat")
                nc.sync.dma_start(xft[:], d_xfbf[b, i * 128:(i + 1) * 128, :])
                st = colp.tile([128, 3, 6], F32, tag="bnstf")
                for g in range(3):
                    nc.vector.bn_stats(st[:, g, :], xft[:, g * 256:(g + 1) * 256])
                nc.vector.bn_aggr(mvf[:, i, :], st[:])
            stdf = colp.tile([128, NT], F32, tag="stdf")
            nc.scalar.activation(stdf[:], mvf[:, :, 1], AF.Sqrt, bias=eps_c[:])
            rstdf = colp.tile([128, NT], F32, tag="rstdf")
            nc.vector.reciprocal(rstdf[:], stdf[:])
            packf = colp.tile([128, 2 * NT], BF16, tag="packf")
            nc.scalar.mul(packf[:, 0:NT], mvf[:, :, 0], -1.0)
            nc.scalar.copy(packf[:, NT:2 * NT], stdf[:])
            ptf = psa.tile([2 * NT, 128], BF16, tag="ptr")
            nc.tensor.transpose(ptf[:], packf[:], ident[:])
            stgf = colp.tile([2 * NT, 128], BF16, tag="stgf")
            nc.vector.tensor_copy(stgf[:], ptf[:])
            rows_f = rowp.tile([1, 2 * N], BF16, tag="rows_f")
            for r in range(2 * NT):
                nc.sync.dma_start(rows_f[0:1, r * 128:(r + 1) * 128],
                                  stgf[r:r + 1, :])
            nmuf_r = rows_f[:, 0:N]
            stdf_r = rows_f[:, N:2 * N]

            # xf^T for K/V lhsT
            xfT = kvp.tile([128, KTD, N], BF16, tag="xfT")
            for j in range(KTD):
                nc.sync.dma_start_transpose(xfT[:, j, :],
                                            d_xfbf[b, :, j * 128:(j + 1) * 128])

            # ---------- K and V ----------
            exp_k = kvp1.tile([128, NT, D], BF16, tag="expk")
            v_sb = kvp1.tile([128, NT, D], BF16, tag="vsb")
            for nt in range(NT):
                for ch in range(NCH):
                    cs = slice(ch * 512, (ch + 1) * 512)
                    ns = slice(nt * 128, (nt + 1) * 128)
                    pk = psq.tile([128, 512], F32, tag="ps")
                    for kt in range(KTD):
                        nc.tensor.matmul(pk[:], xfT[:, kt, ns], wk_sb[:, kt, cs],
                                         start=(kt == 0), stop=False)
                    nc.tensor.matmul(pk[:], nmuf_r[0:1, ns], cwk_r[0:1, cs],
                                     start=False, stop=False)
                    nc.tensor.matmul(pk[:], stdf_r[0:1, ns], bke_r[0:1, cs],
                                     start=False, stop=True)
                    nc.scalar.activation(exp_k[:, nt, cs], pk[:], AF.Exp,
                                         scale=rstdf[:, nt:nt + 1])
                    pv = psq.tile([128, 512], F32, tag="ps")
                    for kt in range(KTD):
                        nc.tensor.matmul(pv[:], xfT[:, kt, ns], wv_sb[:, kt, cs],
                                         start=(kt == 0), stop=False)
                    nc.tensor.matmul(pv[:], nmuf_r[0:1, ns], cwv_r[0:1, cs],
                                     start=False, stop=False)
                    nc.tensor.matmul(pv[:], stdf_r[0:1, ns], bve_r[0:1, cs],
                                     start=False, stop=True)
                    nc.scalar.activation(v_sb[:, nt, cs], pv[:], AF.Identity,
                                         scale=rstdf[:, nt:nt + 1])

            # ---------- S_k and attn ----------
            pks = psa.tile([128, KD], F32, tag="skattn")
            for j in range(KD):
                for nt in range(NT):
                    nc.tensor.matmul(pks[:, j:j + 1],
                                     exp_k[:, nt, j * 128:(j + 1) * 128],
                                     ones_c[:], start=(nt == 0), stop=(nt == 1))
            r_k = colp.tile([128, KD], F32, tag="rk")
            nc.vector.reciprocal(r_k[:], pks[:])

            patt = psa.tile([128, 512], F32, tag="skattn")
            for h in range(H):
                rp = slice((h % 2) * 64, (h % 2) * 64 + 64)
                cp = slice((h // 2) * 64, (h // 2) * 64 + 64)
                hs = slice(h * 64, (h + 1) * 64)
                for nt in range(NT):
                    nc.tensor.matmul(patt[rp, cp], exp_k[:, nt, hs],
                                     v_sb[:, nt, hs],
                                     start=(nt == 0), stop=(nt == 1))
            # block-diagonal per head pair: [0:64,0:64]=head 2j, [64:,64:]=head 2j+1
            attn_s = kvp.tile([128, KD, 128], BF16, tag="attns")
            nc.vector.memset(attn_s[:], 0.0)
            for j in range(KD):
                nc.vector.tensor_scalar_mul(attn_s[0:64, j, 0:64],
                                            patt[0:64, j * 64:(j + 1) * 64],
                                            r_k[0:64, j:j + 1])
                nc.vector.tensor_scalar_mul(attn_s[64:128, j, 64:128],
                                            patt[64:128, j * 64:(j + 1) * 64],
                                            r_k[64:128, j:j + 1])

            # ---------- x path: stats ----------
            if K_STAGES < 4:
                continue
            mvx = colp.tile([128, TI, 2], F32, tag="mvx")
            for i in range(TI):
                xt_ = xstat.tile([128, D], BF16, tag="xstat")
                nc.sync.dma_start(xt_[:], d_xbf[b, i * 128:(i + 1) * 128, :])
                st = colp.tile([128, 2, 6], F32, tag="bnstx")
                for g in range(2):
                    nc.vector.bn_stats(st[:, g, :], xt_[:, g * 512:(g + 1) * 512])
                nc.vector.bn_aggr(mvx[:, i, :], st[:])
            stdx = colp.tile([128, TI], F32, tag="stdx")
            nc.scalar.activation(stdx[:], mvx[:, :, 1], AF.Sqrt, bias=eps_c[:])
            rstdx = colp.tile([128, TI], F32, tag="rstdx")
            nc.vector.reciprocal(rstdx[:], stdx[:])
            packx = colp.tile([128, 2 * TI], BF16, tag="packx")
            nc.scalar.mul(packx[:, 0:TI], mvx[:, :, 0], -1.0)
            nc.scalar.copy(packx[:, TI:2 * TI], stdx[:])
            ptx = psa.tile([2 * TI, 128], BF16, tag="ptr")
            nc.tensor.transpose(ptx[:], packx[:], ident[:])
            stgx = colp.tile([2 * TI, 128], BF16, tag="stgx")
            nc.vector.tensor_copy(stgx[:], ptx[:])
            rows_x = rowp.tile([1, 2 * T], BF16, tag="rows_x")
            for r in range(2 * TI):
                nc.sync.dma_start(rows_x[0:1, r * 128:(r + 1) * 128],
                                  stgx[r:r + 1, :])
            nmux_r = rows_x[:, 0:T]
            stdx_r = rows_x[:, T:2 * T]

            # x^T (raw, bf16) for Q lhsT
            xT = xtp.tile([128, KD, T], BF16, tag="xT")
            for j in range(KD):
                nc.sync.dma_start_transpose(xT[:, j, :],
                                            d_xbf[b, :, j * 128:(j + 1) * 128])

            # ---------- per-t-tile: Q -> softmax -> y -> LN/FiLM/silu ----------
            if K_STAGES < 5:
                continue
            hT = htp.tile([128, KD, T], BF16, tag="hT")
            for ti in range(TI):
                ts_ = slice(ti * 128, (ti + 1) * 128)
                exp_q = qp.tile([128, D], BF16, tag="expq")
                for ch in range(NCH):
                    cs = slice(ch * 512, (ch + 1) * 512)
                    pq = psq.tile([128, 512], F32, tag="ps")
                    for kt in range(KD):
                        nc.tensor.matmul(pq[:], xT[:, kt, ts_], wq_sb[:, kt, cs],
                                         start=(kt == 0), stop=False)
                    nc.tensor.matmul(pq[:], nmux_r[0:1, ts_], cwq_r[0:1, cs],
                                     start=False, stop=False)
                    nc.tensor.matmul(pq[:], stdx_r[0:1, ts_], bqe_r[0:1, cs],
                                     start=False, stop=True)
                    nc.scalar.activation(exp_q[:, cs], pq[:], AF.Exp,
                                         scale=rstdx[:, ti:ti + 1])
                s_q = colp.tile([128, H], F32, tag="sq")
                nc.vector.reduce_sum(s_q[:],
                                     exp_q[:].rearrange("p (h c) -> p h c", c=C),
                                     axis=mybir.AxisListType.X)
                r_qf = colp.tile([128, H], F32, tag="rqf")
                nc.vector.reciprocal(r_qf[:], s_q[:])
                r_qb = colp.tile([128, H], BF16, tag="rqb")
                nc.scalar.copy(r_qb[:], r_qf[:])
                qsoft = qp.tile([128, D], BF16, tag="qsoft")
                rq_bc = bass.AP(tensor=r_qb.tensor, offset=r_qb.offset,
                                ap=[[r_qb.ap[0][0], 128], [1, H], [0, C]])
                nc.vector.tensor_mul(
                    qsoft[:].rearrange("p (h c) -> p h c", c=C),
                    exp_q[:].rearrange("p (h c) -> p h c", c=C), rq_bc)
                qT = qtp.tile([128, KD, 128], BF16, tag="qT")
                for j in range(KD):
                    nc.sync.dma_start_transpose(qT[:, j, :],
                                                qsoft[:, j * 128:(j + 1) * 128])
                if K_STAGES < 6:
                    continue
                # y (natural layout) per head
                pys = []
                for ch in range(NCH):
                    py = psy.tile([128, 512], F32, tag="py")
                    pys.append(py)
                for j in range(KD):
                    nc.tensor.matmul(
                        pys[j // 4][:, (j % 4) * 128:(j % 4) * 128 + 128],
                        qT[:, j, :], attn_s[:, j, :],
                        start=True, stop=True)
                if K_SUB < 2:
                    dmy = hp.tile([128, D], BF16, tag="dmy")
                    nc.vector.tensor_copy(dmy[:, 0:512], pys[0][:])
                    nc.vector.tensor_copy(dmy[:, 512:1024], pys[1][:])
                    continue
                # LN(y) stats
                sty = colp.tile([128, 2, 6], F32, tag="bnsty")
                nc.vector.bn_stats(sty[:, 0, :], pys[0][:])
                nc.vector.bn_stats(sty[:, 1, :], pys[1][:])
                mvy = colp.tile([128, 2], F32, tag="mvy")
                nc.vector.bn_aggr(mvy[:], sty[:])
                stdy = colp.tile([128, 1], F32, tag="stdy")
                nc.scalar.activation(stdy[:], mvy[:, 1:2], AF.Sqrt, bias=eps_c[:])
                rstdy = colp.tile([128, 1], F32, tag="rstdy")
                nc.vector.reciprocal(rstdy[:], stdy[:])
                nmry = colp.tile([128, 1], F32, tag="nmry")
                nc.vector.scalar_tensor_tensor(nmry[:], mvy[:, 0:1], -1.0,
                                               rstdy[:], op0=ALU.mult,
                                               op1=ALU.mult)
                if K_SUB < 3:
                    continue
                silu_h = hp.tile([128, D], BF16, tag="siluh")
                for ch in range(NCH):
                    cs = slice(ch * 512, (ch + 1) * 512)
                    stdt = hp.tile([128, 512], BF16, tag="stdt")
                    nc.scalar.activation(stdt[:], pys[ch][:], AF.Identity,
                                         bias=nmry[:], scale=rstdy[:])
                    if K_SUB < 4:
                        nc.vector.tensor_copy(silu_h[:, cs], stdt[:])
                        continue
                    nc.vector.tensor_mul(stdt[:], stdt[:], a_bc[:, cs])
                    nc.vector.tensor_add(stdt[:], stdt[:], b_bc[:, cs])
                    sgt = hp.tile([128, 512], BF16, tag="sgt")
                    nc.scalar.activation(sgt[:], stdt[:], AF.Sigmoid)
                    nc.vector.tensor_mul(silu_h[:, cs], stdt[:], sgt[:])
                if K_STAGES < 7:
                    continue
                for j in range(KD):
                    nc.sync.dma_start_transpose(hT[:, j, ts_],
                                                silu_h[:, j * 128:(j + 1) * 128])

            # ---------- out projection + residual ----------
            if K_STAGES < 7:
                continue
            for ti in range(TI):
                ts_ = slice(ti * 128, (ti + 1) * 128)
                xr = resp.tile([128, D], F32, tag="xr")
                nc.sync.dma_start(xr[:], d_xf32[b, ts_, :])
                for ch in range(NCH):
                    cs = slice(ch * 512, (ch + 1) * 512)
                    po = pso.tile([128, 512], F32, tag="po")
                    for j in range(KD):
                        nc.tensor.matmul(po[:], hT[:, j, ts_], wo_sb[:, j, cs],
                                         start=(j == 0), stop=False)
                    nc.tensor.matmul(po[:], ones_r[0:1, 0:128], outb_r[0:1, cs],
                                     start=False, stop=True)
                    o_sb = outp.tile([128, 512], F32, tag="osb")
                    nc.vector.tensor_add(o_sb[:], po[:], xr[:, cs])
                    nc.sync.dma_start(d_out[b, ts_, cs], o_sb[:])

    nc.compile()
    return nc


def _get_program():
    global _PROGRAM
    if _PROGRAM is None:
        _PROGRAM = _build_program()
    return _PROGRAM


def _prep_inputs(inputs):
    f = lambda k: np.asarray(inputs[k], np.float32)
    x, xf, emb = f("x"), f("xf"), f("emb")
    norm_g, norm_b = f("norm_g"), f("norm_b")
    tnorm_g, tnorm_b = f("tnorm_g"), f("tnorm_b")
    Wq, bq, Wk, bk, Wv, bv = f("Wq"), f("bq"), f("Wk"), f("bk"), f("Wv"), f("bv")
    emb_W, emb_b = f("emb_W"), f("emb_b")
    fg, fb = f("fnorm_g"), f("fnorm_b")
    out_W, out_b = f("out_W"), f("out_b")

    wq_e = norm_g[:, None] * Wq
    wk_e = tnorm_g[:, None] * Wk
    wv_e = tnorm_g[:, None] * Wv
    shared = {
        "wq": wq_e.astype(NBF), "wk": wk_e.astype(NBF), "wv": wv_e.astype(NBF),
        "wo": out_W.astype(NBF), "wemb": emb_W.astype(NBF),
        "cwq": wq_e.sum(0).astype(NBF), "cwk": wk_e.sum(0).astype(NBF),
        "cwv": wv_e.sum(0).astype(NBF),
        "bqe": (bq + norm_b @ Wq).astype(NBF),
        "bke": (bk + tnorm_b @ Wk).astype(NBF),
        "bve": (bv + tnorm_b @ Wv).astype(NBF),
        "outb": out_b.astype(NBF), "embb": emb_b.astype(NBF),
        "fg": fg, "fb": fb,
    }
    xbf = x.astype(NBF)
    xfbf = xf.astype(NBF)
    in_maps = []
    for i in range(NCORES):
        s = slice(i * BPC, (i + 1) * BPC)
        m = dict(shared)
        m["xbf"] = xbf[s]
        m["xf32"] = x[s]
        m["xfbf"] = xfbf[s]
        m["emb"] = emb[s]
        in_maps.append(m)
    return in_maps


def run(inputs, trace=False):
    nc = _get_program()
    in_maps = _prep_inputs(inputs)
    res = run_bass_kernel_spmd(nc, in_maps, core_ids=list(range(NCORES)),
                               trace=trace)
    out = np.concatenate([res.results[i]["out"] for i in range(NCORES)], axis=0)
    return out, res


def kernel(**inputs):
    out, _ = run(inputs, trace=False)
    return out



# revision 5
# speedup vs baseline: 1.9932x; 1.9932x over previous
"""Trainium2 Bass kernel for nn_CrossAttention (linear cross-attention block).

Computation (per batch b):
  xn  = LN(x[b]; norm_g, norm_b)                 [T, D]
  xfn = LN(xf[b]; tnorm_g, tnorm_b)              [N, TD]
  q   = softmax_c((xn @ Wq + bq).reshape(T,H,C))
  k   = softmax_n((xfn @ Wk + bk).reshape(N,H,C))
  v   = (xfn @ Wv + bv).reshape(N,H,C)
  attn= einsum('nhc,nhd->hcd', k, v); y = einsum('thc,hcd->thd', q, attn)
  e   = silu(emb) @ emb_W + emb_b; scale, shift = split(e)
  h   = LN(y; fnorm_g, fnorm_b) * (1+scale) + shift
  out = x + silu(h) @ out_W + out_b

Sharding: pure data-parallel over batch B=32 across 8 NeuronCores (4 each).

Device strategy:
  - LN gain folded into projection weights on the host. x / xf are
    normalized on-chip in natural layout (per-partition -mu / 1/std via one
    DVE tensor_scalar per tile), so projections are plain matmuls and the
    exp/copy PSUM drains need no per-row scale. Bias rank-1 folds are only
    emitted when biases are nonzero (they are zero for this model).
  - All transposes via the DMA xbar engine with multi-block destinations:
    one instruction per [128, D] tile ([128, KD, 128] dst), not one per
    128x128 block. This cuts sync-queue DMA occupancy ~8x.
  - t-tile loop is software-pipelined with a 2-deep skew: per iteration the
    PE runs Q-proj(ti), y-matmul(ti-1), out-proj(ti-2) so the exp/softmax/
    transpose and LN/FiLM/silu/transpose tails of each tile hide under the
    next tiles' matmuls and the PE stays warm (HAM K=8/8).
  - Dual softmax: q-softmax over C is a grouped free-dim reduce + broadcast
    multiply; k-softmax over N folds into a per-partition reciprocal scale
    of the (exp_k^T v) head matmuls.
  - Residual uses the raw bf16 x kept resident in SBUF; output is stored
    bf16 and upcast on the host.
"""

from contextlib import ExitStack

import numpy as np
import ml_dtypes

import concourse.bass as bass
import concourse.mybir as mybir
import concourse.tile as tile
from concourse import bacc, library_config
from concourse.bass_utils import run_bass_kernel_spmd
from concourse.masks import make_identity

# problem shapes (hardcoded per contract)
B, T, N, D, TD, H, C, TE = 32, 1024, 256, 1024, 768, 16, 64, 2048
D2 = 2 * D
EPS = 1e-5
NCORES = 8
BPC = B // NCORES           # batches per core
TI = T // 128               # 8 t-tiles
KD = D // 128               # 8 k-tiles over D
KTD = TD // 128             # 6 k-tiles over TD
KTE = TE // 128             # 16 k-tiles over TE
NT = N // 128               # 2 n-tiles
NCH = D // 512              # 2 free 512-chunks over D

F32 = mybir.dt.float32
BF16 = mybir.dt.bfloat16
AF = mybir.ActivationFunctionType
ALU = mybir.AluOpType
NBF = ml_dtypes.bfloat16

_PROGRAMS = {}  # cached (nc) builds keyed by bias flags


def _build_program(qkv_bias, out_bias):
    nc = bacc.Bacc("TRN2", target_bir_lowering=False, debug=False,
                   num_devices=NCORES)

    # ---- DRAM I/O ----
    d_xbf = nc.dram_tensor("xbf", [BPC, T, D], BF16, kind="ExternalInput")
    d_xfbf = nc.dram_tensor("xfbf", [BPC, N, TD], BF16, kind="ExternalInput")
    d_emb = nc.dram_tensor("emb", [BPC, TE], F32, kind="ExternalInput")
    d_wq = nc.dram_tensor("wq", [D, D], BF16, kind="ExternalInput")
    d_wk = nc.dram_tensor("wk", [TD, D], BF16, kind="ExternalInput")
    d_wv = nc.dram_tensor("wv", [TD, D], BF16, kind="ExternalInput")
    d_wo = nc.dram_tensor("wo", [D, D], BF16, kind="ExternalInput")
    d_wemb = nc.dram_tensor("wemb", [TE, D2], BF16, kind="ExternalInput")
    d_bqe = nc.dram_tensor("bqe", [D], BF16, kind="ExternalInput")
    d_bke = nc.dram_tensor("bke", [D], BF16, kind="ExternalInput")
    d_bve = nc.dram_tensor("bve", [D], BF16, kind="ExternalInput")
    d_outb = nc.dram_tensor("outb", [D], BF16, kind="ExternalInput")
    d_embb = nc.dram_tensor("embb", [D2], BF16, kind="ExternalInput")
    d_fg = nc.dram_tensor("fg", [D], F32, kind="ExternalInput")
    d_fb = nc.dram_tensor("fb", [D], F32, kind="ExternalInput")
    d_out = nc.dram_tensor("out", [BPC, T, D], BF16, kind="ExternalOutput")

    with tile.TileContext(nc) as tc, ExitStack() as ctx:
        wpool = ctx.enter_context(tc.tile_pool(name="weights", bufs=1))
        cpool = ctx.enter_context(tc.tile_pool(name="consts", bufs=1))

        # ---- persistent weights ----
        wq_sb = wpool.tile([128, KD, D], BF16)
        nc.sync.dma_start(wq_sb[:], d_wq[:].rearrange("(i p) m -> p i m", p=128))
        wk_sb = wpool.tile([128, KTD, D], BF16)
        nc.sync.dma_start(wk_sb[:], d_wk[:].rearrange("(i p) m -> p i m", p=128))
        wv_sb = wpool.tile([128, KTD, D], BF16)
        nc.sync.dma_start(wv_sb[:], d_wv[:].rearrange("(i p) m -> p i m", p=128))
        wo_sb = wpool.tile([128, KD, D], BF16)
        nc.sync.dma_start(wo_sb[:], d_wo[:].rearrange("(i p) m -> p i m", p=128))

        # ---- constants ----
        def row_const(dram, n, tag):
            t = cpool.tile([1, n], BF16, tag=tag)
            nc.sync.dma_start(t[:], dram[None, :])
            return t
        bqe_r = row_const(d_bqe, D, "bqe_r") if qkv_bias else None
        bke_r = row_const(d_bke, D, "bke_r") if qkv_bias else None
        bve_r = row_const(d_bve, D, "bve_r") if qkv_bias else None
        outb_r = row_const(d_outb, D, "outb_r") if out_bias else None
        ones_r = cpool.tile([1, 128], BF16)
        nc.vector.memset(ones_r[:], 1.0)
        ones_c = cpool.tile([128, 1], BF16)
        nc.vector.memset(ones_c[:], 1.0)
        eps_c = cpool.tile([128, 1], F32)
        nc.vector.memset(eps_c[:], EPS)
        ident = cpool.tile([128, 128], BF16)
        make_identity(nc, ident[:])
        nc.gpsimd.load_library(library_config.attnmlp)

        a_bf = cpool.tile([BPC, D], BF16)   # FiLM A rows (one per batch)
        b_bf = cpool.tile([BPC, D], BF16)   # FiLM B rows

        # ---- emb / FiLM phase (all 4 batches at once) ----
        with tc.tile_pool(name="wemb", bufs=1) as wep, \
             tc.tile_pool(name="etmp", bufs=1) as ep, \
             tc.tile_pool(name="pse", bufs=2, space=bass.MemorySpace.PSUM) as pse:
            wemb_sb = wep.tile([128, KTE, D2], BF16)
            embb_r = ep.tile([1, D2], BF16)
            nc.sync.dma_start(embb_r[:], d_embb[None, :])
            fg_r = ep.tile([1, D], F32)
            nc.sync.dma_start(fg_r[:], d_fg[None, :])
            fb_r = ep.tile([1, D], F32)
            nc.sync.dma_start(fb_r[:], d_fb[None, :])
            nc.sync.dma_start(wemb_sb[:],
                              d_wemb[:].rearrange("(i p) m -> p i m", p=128))
            emb_sb = ep.tile([BPC, TE], F32)
            nc.sync.dma_start(emb_sb[:], d_emb[:])
            semb = ep.tile([BPC, TE], BF16)
            nc.scalar.activation(semb[:], emb_sb[:], AF.Silu)
            embT = ep.tile([128, KTE, BPC], BF16)
            for c in range(KTE):
                pst = pse.tile([128, BPC], BF16, tag="pst")
                nc.tensor.transpose(pst[:], semb[:, c * 128:(c + 1) * 128],
                                    ident[0:BPC, 0:BPC])
                nc.vector.tensor_copy(embT[:, c, :], pst[:])
            e_sb = ep.tile([BPC, D2], F32)
            for ch in range(D2 // 512):
                pe = pse.tile([BPC, 512], F32, tag="pe")
                for kt in range(KTE):
                    nc.tensor.matmul(pe[:], embT[:, kt, :],
                                     wemb_sb[:, kt, ch * 512:(ch + 1) * 512],
                                     start=(kt == 0), stop=False)
                nc.tensor.matmul(pe[:], ones_r[0:1, 0:BPC],
                                 embb_r[0:1, ch * 512:(ch + 1) * 512],
                                 start=False, stop=True)
                nc.vector.tensor_copy(e_sb[:, ch * 512:(ch + 1) * 512], pe[:])
            # FiLM rows: A = fg*(1+scale), Bf = fb*(1+scale) + shift
            fg4 = ep.tile([BPC, D], F32)
            nc.gpsimd.partition_broadcast(fg4[:], fg_r[:], channels=BPC)
            fb4 = ep.tile([BPC, D], F32)
            nc.gpsimd.partition_broadcast(fb4[:], fb_r[:], channels=BPC)
            tall = ep.tile([BPC, D], F32)
            nc.vector.tensor_scalar_add(tall[:], e_sb[:, 0:D], 1.0)
            nc.vector.tensor_mul(a_bf[:], tall[:], fg4[:])
            btmp = ep.tile([BPC, D], F32)
            nc.vector.tensor_mul(btmp[:], tall[:], fb4[:])
            nc.vector.tensor_add(b_bf[:], btmp[:], e_sb[:, D:D2])

        # ---- batch-phase pools ----
        xp = ctx.enter_context(tc.tile_pool(name="x", bufs=2))
        xfp = ctx.enter_context(tc.tile_pool(name="xf", bufs=2))
        statp = ctx.enter_context(tc.tile_pool(name="stat", bufs=2))
        kvp = ctx.enter_context(tc.tile_pool(name="kv", bufs=2))
        xnp = ctx.enter_context(tc.tile_pool(name="xn", bufs=2))
        xntp = ctx.enter_context(tc.tile_pool(name="xnT", bufs=2))
        qp = ctx.enter_context(tc.tile_pool(name="q", bufs=2))
        qtp = ctx.enter_context(tc.tile_pool(name="qT", bufs=2))
        hp = ctx.enter_context(tc.tile_pool(name="h", bufs=2))
        htp = ctx.enter_context(tc.tile_pool(name="hT", bufs=3))
        outp = ctx.enter_context(tc.tile_pool(name="o", bufs=2))
        abp = ctx.enter_context(tc.tile_pool(name="ab", bufs=2))
        colp = ctx.enter_context(tc.tile_pool(name="cols", bufs=2))
        psq = ctx.enter_context(
            tc.tile_pool(name="psq", bufs=2, space=bass.MemorySpace.PSUM))
        psy = ctx.enter_context(
            tc.tile_pool(name="psy", bufs=2, space=bass.MemorySpace.PSUM))
        pso = ctx.enter_context(
            tc.tile_pool(name="pso", bufs=2, space=bass.MemorySpace.PSUM))
        psa = ctx.enter_context(
            tc.tile_pool(name="psa", bufs=1, space=bass.MemorySpace.PSUM))

        for b in range(BPC):
            # FiLM broadcast rows for this batch
            arow = abp.tile([1, D], BF16, tag="arow")
            nc.sync.dma_start(arow[:], a_bf[b:b + 1, :])
            brow = abp.tile([1, D], BF16, tag="brow")
            nc.sync.dma_start(brow[:], b_bf[b:b + 1, :])
            a_bc = abp.tile([128, D], BF16, tag="abc")
            nc.gpsimd.partition_broadcast(a_bc[:], arow[:], channels=128)
            b_bc = abp.tile([128, D], BF16, tag="bbc")
            nc.gpsimd.partition_broadcast(b_bc[:], brow[:], channels=128)

            # ---------- xf path ----------
            xf_nat = xfp.tile([128, NT, TD], BF16, tag="xfnat")
            nc.sync.dma_start(xf_nat[:],
                              d_xfbf[b].rearrange("(i p) m -> p i m", p=128))
            mvf = statp.tile([128, NT, 2], F32, tag="mvf")
            for i in range(NT):
                st = statp.tile([128, 3, 6], F32, tag="bnstf")
                for g in range(3):
                    nc.vector.bn_stats(st[:, g, :],
                                       xf_nat[:, i, g * 256:(g + 1) * 256])
                nc.vector.bn_aggr(mvf[:, i, :], st[:])
            stdf = statp.tile([128, NT], F32, tag="stdf")
            nc.scalar.activation(stdf[:], mvf[:, :, 1], AF.Sqrt, bias=eps_c[:])
            rstdf = statp.tile([128, NT], F32, tag="rstdf")
            nc.vector.reciprocal(rstdf[:], stdf[:])
            nmuf = statp.tile([128, NT], F32, tag="nmuf")
            nc.scalar.mul(nmuf[:], mvf[:, :, 0], -1.0)
            xfnT = kvp.tile([128, NT, KTD, 128], BF16, tag="xfnT")
            for i in range(NT):
                xfn_t = xfp.tile([128, TD], BF16, tag="xfn")
                nc.vector.tensor_scalar(xfn_t[:], xf_nat[:, i, :],
                                        nmuf[:, i:i + 1], rstdf[:, i:i + 1],
                                        op0=ALU.add, op1=ALU.mult)
                nc.sync.dma_start_transpose(xfnT[:, i], xfn_t[:])

            # ---------- K and V ----------
            exp_k = kvp.tile([128, NT, D], BF16, tag="expk")
            v_sb = kvp.tile([128, NT, D], BF16, tag="vsb")
            for nt in range(NT):
                for ch in range(NCH):
                    cs = slice(ch * 512, (ch + 1) * 512)
                    pk = psq.tile([128, 512], F32, tag="ps")
                    for kt in range(KTD):
                        nc.tensor.matmul(pk[:], xfnT[:, nt, kt, :],
                                         wk_sb[:, kt, cs],
                                         start=(kt == 0),
                                         stop=(kt == KTD - 1 and not qkv_bias))
                    if qkv_bias:
                        nc.tensor.matmul(pk[:], ones_r[:], bke_r[0:1, cs],
                                         start=False, stop=True)
                    nc.scalar.activation(exp_k[:, nt, cs], pk[:], AF.Exp)
                    pv = psq.tile([128, 512], F32, tag="ps")
                    for kt in range(KTD):
                        nc.tensor.matmul(pv[:], xfnT[:, nt, kt, :],
                                         wv_sb[:, kt, cs],
                                         start=(kt == 0),
                                         stop=(kt == KTD - 1 and not qkv_bias))
                    if qkv_bias:
                        nc.tensor.matmul(pv[:], ones_r[:], bve_r[0:1, cs],
                                         start=False, stop=True)
                    nc.scalar.copy(v_sb[:, nt, cs], pv[:])

            # ---------- S_k and attn ----------
            pks = psa.tile([128, KD], F32, tag="pks")
            for j in range(KD):
                for nt in range(NT):
                    nc.tensor.matmul(pks[:, j:j + 1],
                                     exp_k[:, nt, j * 128:(j + 1) * 128],
                                     ones_c[:], start=(nt == 0), stop=(nt == 1))
            r_k = statp.tile([128, KD], F32, tag="rk")
            nc.vector.reciprocal(r_k[:], pks[:])

            patt = psa.tile([128, 512], F32, tag="patt")
            for h in range(H):
                rp = slice((h % 2) * 64, (h % 2) * 64 + 64)
                cp = slice((h // 2) * 64, (h // 2) * 64 + 64)
                hs = slice(h * 64, (h + 1) * 64)
                for nt in range(NT):
                    nc.tensor.matmul(patt[rp, cp], exp_k[:, nt, hs],
                                     v_sb[:, nt, hs],
                                     start=(nt == 0), stop=(nt == 1))
            # block-diagonal per head pair: [0:64,0:64]=head 2j, [64:,64:]=2j+1
            attn_s = kvp.tile([128, KD, 128], BF16, tag="attns")
            nc.vector.memset(attn_s[:], 0.0)
            for j in range(KD):
                nc.vector.tensor_scalar_mul(attn_s[0:64, j, 0:64],
                                            patt[0:64, j * 64:(j + 1) * 64],
                                            r_k[0:64, j:j + 1])
                nc.vector.tensor_scalar_mul(attn_s[64:128, j, 64:128],
                                            patt[64:128, j * 64:(j + 1) * 64],
                                            r_k[64:128, j:j + 1])

            # ---------- x path: load + stats ----------
            x_nat = xp.tile([128, TI, D], BF16, tag="xnat")
            nc.sync.dma_start(x_nat[:],
                              d_xbf[b].rearrange("(i p) m -> p i m", p=128))
            mvx = statp.tile([128, TI, 2], F32, tag="mvx")
            for i in range(TI):
                st = statp.tile([128, 2, 6], F32, tag="bnstx")
                for g in range(2):
                    nc.vector.bn_stats(st[:, g, :],
                                       x_nat[:, i, g * 512:(g + 1) * 512])
                nc.vector.bn_aggr(mvx[:, i, :], st[:])
            stdx = statp.tile([128, TI], F32, tag="stdx")
            nc.scalar.activation(stdx[:], mvx[:, :, 1], AF.Sqrt, bias=eps_c[:])
            rstdx = statp.tile([128, TI], F32, tag="rstdx")
            nc.vector.reciprocal(rstdx[:], stdx[:])
            nmux = statp.tile([128, TI], F32, tag="nmux")
            nc.scalar.mul(nmux[:], mvx[:, :, 0], -1.0)

            # ---------- software-pipelined t-tile loop ----------
            # per iteration: prep xnT(ti+1) | PE: Q(ti), y(ti-1), out(ti-2)
            xnT_tiles = {}
            qT_tiles = {}
            hT_tiles = {}
            py_tiles = {}

            # prologue: xnT(0)
            xn_t = xnp.tile([128, D], BF16, tag="xn")
            nc.vector.tensor_scalar(xn_t[:], x_nat[:, 0, :], nmux[:, 0:1],
                                    rstdx[:, 0:1], op0=ALU.add, op1=ALU.mult)
            xnT_tiles[0] = xntp.tile([128, KD, 128], BF16, tag="xnT", name="xnT0")
            nc.sync.dma_start_transpose(xnT_tiles[0][:], xn_t[:])

            for it in range(TI + 2):
                # ---- prep xn/xnT for tile it+1 ----
                if it + 1 < TI:
                    xn_t = xnp.tile([128, D], BF16, tag="xn")
                    nc.vector.tensor_scalar(
                        xn_t[:], x_nat[:, it + 1, :], nmux[:, it + 1:it + 2],
                        rstdx[:, it + 1:it + 2], op0=ALU.add, op1=ALU.mult)
                    xnT_tiles[it + 1] = xntp.tile([128, KD, 128], BF16,
                                                  tag="xnT", name="xnTt")
                    nc.sync.dma_start_transpose(xnT_tiles[it + 1][:], xn_t[:])

                # ---- stage A: Q-proj / exp / softmax / qT for tile it ----
                if it < TI:
                    xnT_t = xnT_tiles.pop(it)
                    exp_q = qp.tile([128, D], BF16, tag="expq")
                    s_q = colp.tile([128, H], F32, tag="sq")
                    for ch in range(NCH):
                        cs = slice(ch * 512, (ch + 1) * 512)
                        pq = psq.tile([128, 512], F32, tag="ps")
                        for kt in range(KD):
                            nc.tensor.matmul(
                                pq[:], xnT_t[:, kt, :], wq_sb[:, kt, cs],
                                start=(kt == 0),
                                stop=(kt == KD - 1 and not qkv_bias))
                        if qkv_bias:
                            nc.tensor.matmul(pq[:], ones_r[:], bqe_r[0:1, cs],
                                             start=False, stop=True)
                        nc.scalar.activation(exp_q[:, cs], pq[:], AF.Exp)
                        nc.vector.reduce_sum(
                            s_q[:, ch * 8:(ch + 1) * 8],
                            exp_q[:, cs].rearrange("p (h c) -> p h c", c=C),
                            axis=mybir.AxisListType.X)
                    r_qf = colp.tile([128, H], F32, tag="rqf")
                    nc.vector.reciprocal(r_qf[:], s_q[:])
                    r_qb = colp.tile([128, H], BF16, tag="rqb")
                    nc.scalar.copy(r_qb[:], r_qf[:])
                    qsoft = qp.tile([128, D], BF16, tag="qsoft")
                    rq_bc = bass.AP(tensor=r_qb.tensor, offset=r_qb.offset,
                                    ap=[[r_qb.ap[0][0], 128], [1, H], [0, C]])
                    nc.vector.tensor_mul(
                        qsoft[:].rearrange("p (h c) -> p h c", c=C),
                        exp_q[:].rearrange("p (h c) -> p h c", c=C), rq_bc)
                    qT_tiles[it] = qtp.tile([128, KD, 128], BF16, tag="qT", name="qTt")
                    nc.sync.dma_start_transpose(qT_tiles[it][:], qsoft[:])

                # ---- stage B: y / LN / FiLM / silu / hT for tile it-1 ----
                tj = it - 1
                if 0 <= tj < TI:
                    qT_t = qT_tiles.pop(tj)
                    pys = [psy.tile([128, 512], F32, tag="py", name=f"py{ch}")
                           for ch in range(NCH)]
                    for j in range(KD):
                        nc.tensor.matmul(
                            pys[j // 4][:, (j % 4) * 128:(j % 4) * 128 + 128],
                            qT_t[:, j, :], attn_s[:, j, :],
                            start=True, stop=True)
                    sty = colp.tile([128, 2, 6], F32, tag="bnsty")
                    nc.vector.bn_stats(sty[:, 0, :], pys[0][:])
                    nc.vector.bn_stats(sty[:, 1, :], pys[1][:])
                    mvy = colp.tile([128, 2], F32, tag="mvy")
                    nc.vector.bn_aggr(mvy[:], sty[:])
                    stdy = colp.tile([128, 1], F32, tag="stdy")
                    nc.scalar.activation(stdy[:], mvy[:, 1:2], AF.Sqrt,
                                         bias=eps_c[:])
                    rstdy = colp.tile([128, 1], F32, tag="rstdy")
                    nc.vector.reciprocal(rstdy[:], stdy[:])
                    nmry = colp.tile([128, 1], F32, tag="nmry")
                    nc.vector.scalar_tensor_tensor(nmry[:], mvy[:, 0:1], -1.0,
                                                   rstdy[:], op0=ALU.mult,
                                                   op1=ALU.mult)
                    silu_h = hp.tile([128, D], BF16, tag="siluh")
                    for ch in range(NCH):
                        cs = slice(ch * 512, (ch + 1) * 512)
                        stdt = hp.tile([128, 512], BF16, tag="stdt")
                        nc.scalar.activation(stdt[:], pys[ch][:], AF.Identity,
                                             bias=nmry[:], scale=rstdy[:])
                        film = hp.tile([128, 512], BF16, tag="film")
                        nc.vector.tensor_mul(film[:], stdt[:], a_bc[:, cs])
                        nc.vector.tensor_add(film[:], film[:], b_bc[:, cs])
                        nc.scalar.activation(silu_h[:, cs], film[:], AF.Silu)
                    hT_tiles[tj] = htp.tile([128, KD, 128], BF16, tag="hT", name="hTt")
                    nc.sync.dma_start_transpose(hT_tiles[tj][:], silu_h[:])

                # ---- stage C: out-proj + residual + store for tile it-2 ----
                tk = it - 2
                if tk >= 0:
                    hT_t = hT_tiles.pop(tk)
                    o_sb = outp.tile([128, D], BF16, tag="osb")
                    for ch in range(NCH):
                        cs = slice(ch * 512, (ch + 1) * 512)
                        po = pso.tile([128, 512], F32, tag="po")
                        for j in range(KD):
                            nc.tensor.matmul(
                                po[:], hT_t[:, j, :], wo_sb[:, j, cs],
                                start=(j == 0),
                                stop=(j == KD - 1 and not out_bias))
                        if out_bias:
                            nc.tensor.matmul(po[:], ones_r[:],
                                             outb_r[0:1, cs],
                                             start=False, stop=True)
                        nc.vector.tensor_add(o_sb[:, cs], po[:],
                                             x_nat[:, tk, cs])
                    nc.sync.dma_start(
                        d_out[b, tk * 128:(tk + 1) * 128, :], o_sb[:])

    nc.compile()
    return nc


def _get_program(qkv_bias, out_bias):
    key = (qkv_bias, out_bias)
    if key not in _PROGRAMS:
        _PROGRAMS[key] = _build_program(qkv_bias, out_bias)
    return _PROGRAMS[key]


def _prep_inputs(inputs):
    f = lambda k: np.asarray(inputs[k], np.float32)
    x, xf, emb = f("x"), f("xf"), f("emb")
    norm_g, norm_b = f("norm_g"), f("norm_b")
    tnorm_g, tnorm_b = f("tnorm_g"), f("tnorm_b")
    Wq, bq, Wk, bk, Wv, bv = f("Wq"), f("bq"), f("Wk"), f("bk"), f("Wv"), f("bv")
    emb_W, emb_b = f("emb_W"), f("emb_b")
    fg, fb = f("fnorm_g"), f("fnorm_b")
    out_W, out_b = f("out_W"), f("out_b")

    wq_e = norm_g[:, None] * Wq
    wk_e = tnorm_g[:, None] * Wk
    wv_e = tnorm_g[:, None] * Wv
    bqe = bq + norm_b @ Wq
    bke = bk + tnorm_b @ Wk
    bve = bv + tnorm_b @ Wv
    qkv_bias = bool(np.any(bqe) or np.any(bke) or np.any(bve))
    out_bias = bool(np.any(out_b))
    shared = {
        "wq": wq_e.astype(NBF), "wk": wk_e.astype(NBF), "wv": wv_e.astype(NBF),
        "wo": out_W.astype(NBF), "wemb": emb_W.astype(NBF),
        "bqe": bqe.astype(NBF), "bke": bke.astype(NBF), "bve": bve.astype(NBF),
        "outb": out_b.astype(NBF), "embb": emb_b.astype(NBF),
        "fg": fg, "fb": fb,
    }
    xbf = x.astype(NBF)
    xfbf = xf.astype(NBF)
    in_maps = []
    for i in range(NCORES):
        s = slice(i * BPC, (i + 1) * BPC)
        m = dict(shared)
        m["xbf"] = xbf[s]
        m["xfbf"] = xfbf[s]
        m["emb"] = emb[s]
        in_maps.append(m)
    return in_maps, qkv_bias, out_bias


def run(inputs, trace=False):
    in_maps, qkv_bias, out_bias = _prep_inputs(inputs)
    nc = _get_program(qkv_bias, out_bias)
    res = run_bass_kernel_spmd(nc, in_maps, core_ids=list(range(NCORES)),
                               trace=trace)
    out = np.concatenate(
        [res.results[i]["out"].astype(np.float32) for i in range(NCORES)],
        axis=0)
    return out, res


def kernel(**inputs):
    out, _ = run(inputs, trace=False)
    return out


# revision 12
# speedup vs baseline: 2.2516x; 1.1297x over previous
"""Trainium2 Bass kernel for nn_CrossAttention (linear cross-attention block).

Computation (per batch b):
  xn  = LN(x[b]; norm_g, norm_b)                 [T, D]
  xfn = LN(xf[b]; tnorm_g, tnorm_b)              [N, TD]
  q   = softmax_c((xn @ Wq + bq).reshape(T,H,C))
  k   = softmax_n((xfn @ Wk + bk).reshape(N,H,C))
  v   = (xfn @ Wv + bv).reshape(N,H,C)
  attn= einsum('nhc,nhd->hcd', k, v); y = einsum('thc,hcd->thd', q, attn)
  e   = silu(emb) @ emb_W + emb_b; scale, shift = split(e)
  h   = LN(y; fnorm_g, fnorm_b) * (1+scale) + shift
  out = x + silu(h) @ out_W + out_b

Sharding: pure data-parallel over batch B=32 across 8 NeuronCores (4 each).

Device strategy:
  - LN gain folded into projection weights on the host. x / xf are
    normalized on-chip in natural layout (per-partition -mu / 1/std via one
    DVE tensor_scalar per tile), so projections are plain matmuls and the
    exp/copy PSUM drains need no per-row scale. Bias rank-1 folds are only
    emitted when biases are nonzero (they are zero for this model).
  - The scalar engine stays on ONE activation table set (exp_and_others:
    exp/tanh/identity/copy/square) for the whole kernel: 1/sqrt(var+eps)
    is computed on DVE with the bit-trick + 2 Newton steps, and silu(x) is
    x/2*(1+tanh(x/2)) with the 1/2 folded into the FiLM A/B coefficients
    (host passes fnorm_g/2, fnorm_b/2). This removes ~110 ACT_TABLE_LOADs
    (~1.5us each) that otherwise thrash between exp/sqrt/silu sets.
  - All transposes via the DMA xbar engine with multi-block destinations:
    one instruction per [128, D] tile ([128, KD, 128] dst), not one per
    128x128 block: flat ~1.2us sync-queue cost per instruction.
  - t-tile loop is software-pipelined with a 2-deep skew: per iteration the
    PE runs Q-proj(ti), y-matmul(ti-1), out-proj(ti-2) so the exp/softmax/
    transpose and LN/FiLM/silu/transpose tails of each tile hide under the
    next tiles' matmuls and the PE stays warm (HAM K=8/8). The next batch's
    x/xf loads + LN stats are spread across the current batch's t-loop.
  - Dual softmax: q-softmax over C is a grouped free-dim reduce + broadcast
    multiply; k-softmax over N folds into a per-partition reciprocal scale
    of the (exp_k^T v) head matmuls.
  - Residual uses the raw bf16 x kept resident in SBUF (ACT drains the out
    PSUM, DVE adds at bf16 2x rate); output stored bf16, upcast on host.
"""

from contextlib import ExitStack

import numpy as np
import ml_dtypes

import concourse.bass as bass
import concourse.mybir as mybir
import concourse.tile as tile
from concourse import bacc, library_config
from concourse.bass_utils import run_bass_kernel_spmd
from concourse.masks import make_identity

# problem shapes (hardcoded per contract)
B, T, N, D, TD, H, C, TE = 32, 1024, 256, 1024, 768, 16, 64, 2048
D2 = 2 * D
EPS = 1e-5
NCORES = 8
BPC = B // NCORES           # batches per core
TI = T // 128               # 8 t-tiles
KD = D // 128               # 8 k-tiles over D
KTD = TD // 128             # 6 k-tiles over TD
KTE = TE // 128             # 16 k-tiles over TE
NT = N // 128               # 2 n-tiles
NCH = D // 512              # 2 free 512-chunks over D

F32 = mybir.dt.float32
BF16 = mybir.dt.bfloat16
I32 = mybir.dt.int32
AF = mybir.ActivationFunctionType
ALU = mybir.AluOpType
NBF = ml_dtypes.bfloat16
MAGIC = 0x5F3759DF

_PROGRAMS = {}  # cached (nc) builds keyed by bias flags


def _build_program(qkv_bias, out_bias):
    nc = bacc.Bacc("TRN2", target_bir_lowering=False, debug=False,
                   num_devices=NCORES)

    # ---- DRAM I/O ----
    d_xbf = nc.dram_tensor("xbf", [BPC, T, D], BF16, kind="ExternalInput")
    d_xfbf = nc.dram_tensor("xfbf", [BPC, N, TD], BF16, kind="ExternalInput")
    d_emb = nc.dram_tensor("emb", [BPC, TE], BF16, kind="ExternalInput")
    d_wq = nc.dram_tensor("wq", [D, D], BF16, kind="ExternalInput")
    d_wk = nc.dram_tensor("wk", [TD, D], BF16, kind="ExternalInput")
    d_wv = nc.dram_tensor("wv", [TD, D], BF16, kind="ExternalInput")
    d_wo = nc.dram_tensor("wo", [D, D], BF16, kind="ExternalInput")
    d_wemb = nc.dram_tensor("wemb", [TE, D2], BF16, kind="ExternalInput")
    d_bqe = nc.dram_tensor("bqe", [D], BF16, kind="ExternalInput")
    d_bke = nc.dram_tensor("bke", [D], BF16, kind="ExternalInput")
    d_bve = nc.dram_tensor("bve", [D], BF16, kind="ExternalInput")
    d_outb = nc.dram_tensor("outb", [D], BF16, kind="ExternalInput")
    d_embb = nc.dram_tensor("embb", [D2], BF16, kind="ExternalInput")
    d_fg = nc.dram_tensor("fg", [D], BF16, kind="ExternalInput")   # fnorm_g/2
    d_fb = nc.dram_tensor("fb", [D], BF16, kind="ExternalInput")   # fnorm_b/2
    d_out = nc.dram_tensor("out", [BPC, T, D], BF16, kind="ExternalOutput")

    with tile.TileContext(nc) as tc, ExitStack() as ctx:
        wpool = ctx.enter_context(tc.tile_pool(name="weights", bufs=1))
        cpool = ctx.enter_context(tc.tile_pool(name="consts", bufs=1))

        # ---- persistent weights (K/V first so batch-0 KV starts early) ----
        wk_sb = wpool.tile([128, KTD, D], BF16)
        nc.sync.dma_start(wk_sb[:], d_wk[:].rearrange("(i p) m -> p i m", p=128))
        wv_sb = wpool.tile([128, KTD, D], BF16)
        nc.sync.dma_start(wv_sb[:], d_wv[:].rearrange("(i p) m -> p i m", p=128))

        # ---- constants ----
        def row_const(dram, n, tag):
            t = cpool.tile([1, n], BF16, tag=tag)
            nc.sync.dma_start(t[:], dram[None, :])
            return t
        bqe_r = row_const(d_bqe, D, "bqe_r") if qkv_bias else None
        bke_r = row_const(d_bke, D, "bke_r") if qkv_bias else None
        bve_r = row_const(d_bve, D, "bve_r") if qkv_bias else None
        outb_r = row_const(d_outb, D, "outb_r") if out_bias else None
        ones_r = cpool.tile([1, 128], BF16)
        nc.vector.memset(ones_r[:], 1.0)
        ones_c = cpool.tile([128, 1], BF16)
        nc.vector.memset(ones_c[:], 1.0)
        magic_c = cpool.tile([128, TI], I32)
        nc.vector.memset(magic_c[:], MAGIC)
        ident = cpool.tile([128, 128], BF16)
        make_identity(nc, ident[:])
        nc.gpsimd.load_library(library_config.attnmlp)

        a_bf = cpool.tile([BPC, D], BF16)   # FiLM A/2 rows (one per batch)
        b_bf = cpool.tile([BPC, D], BF16)   # FiLM B/2 rows

        # ---- batch-phase pools (prep + KV only; t-loop pools open after
        # the emb phase releases its scratch) ----
        xp = ctx.enter_context(tc.tile_pool(name="x", bufs=2))
        xfp = ctx.enter_context(tc.tile_pool(name="xf", bufs=2))
        statp = ctx.enter_context(tc.tile_pool(name="stat", bufs=2))
        kvp = ctx.enter_context(tc.tile_pool(name="kv", bufs=2))
        abp = ctx.enter_context(tc.tile_pool(name="ab", bufs=2))
        colp = ctx.enter_context(tc.tile_pool(name="cols", bufs=2))

        def emit_rsqrt(out, var_src, w, tag):
            """out[128, w] f32 = 1/sqrt(var_src + EPS).

            DVE-only (bit-trick + 2 Newton steps) so the ACT engine never
            has to swap to the sqrt table set."""
            ve = colp.tile([128, w], F32, tag=tag + "ve", name=tag + "ve")
            nc.vector.tensor_scalar_add(ve[:], var_src, EPS)
            t1 = colp.tile([128, w], I32, tag=tag + "t1", name=tag + "t1")
            nc.vector.tensor_scalar(t1[:], ve[:].bitcast(I32), 1, None,
                                    op0=ALU.arith_shift_right)
            nc.vector.scalar_tensor_tensor(out.bitcast(I32), magic_c[:, :w],
                                           0, t1[:], op0=ALU.add,
                                           op1=ALU.subtract)
            aa = colp.tile([128, w], F32, tag=tag + "aa", name=tag + "aa")
            cc = colp.tile([128, w], F32, tag=tag + "cc", name=tag + "cc")
            for _ in range(2):
                nc.vector.tensor_mul(aa[:], ve[:], out)
                nc.vector.tensor_mul(aa[:], aa[:], out)
                nc.vector.tensor_scalar(cc[:], aa[:], -0.5, 1.5,
                                        op0=ALU.mult, op1=ALU.add)
                nc.vector.tensor_mul(out, out, cc[:])

        # ---------- per-batch prep, split into chunks for pipelining ------
        def prep_ab(st, b):
            # must be emitted after the emb phase has written a_bf/b_bf --
            # Tile dependencies only flow forward in trace order
            arow = abp.tile([1, D], BF16, tag="arow", name="arow")
            nc.sync.dma_start(arow[:], a_bf[b:b + 1, :])
            brow = abp.tile([1, D], BF16, tag="brow", name="brow")
            nc.sync.dma_start(brow[:], b_bf[b:b + 1, :])
            st["a_bc"] = abp.tile([128, D], BF16, tag="abc", name="abc")
            nc.gpsimd.partition_broadcast(st["a_bc"][:], arow[:], channels=128)
            st["b_bc"] = abp.tile([128, D], BF16, tag="bbc", name="bbc")
            nc.gpsimd.partition_broadcast(st["b_bc"][:], brow[:], channels=128)

        def prep_loads(b):
            st = {}
            st["xf_nat"] = xfp.tile([128, NT, TD], BF16, tag="xfnat",
                                    name="xfnat")
            nc.sync.dma_start(st["xf_nat"][:],
                              d_xfbf[b].rearrange("(i p) m -> p i m", p=128))
            st["x_nat"] = xp.tile([128, TI, D], BF16, tag="xnat", name="xnat")
            nc.sync.dma_start(st["x_nat"][:],
                              d_xbf[b].rearrange("(i p) m -> p i m", p=128))
            return st

        def prep_xf(st):
            xf_nat = st["xf_nat"]
            mvf = statp.tile([128, NT, 2], F32, tag="mvf", name="mvf")
            for i in range(NT):
                bst = statp.tile([128, 3, 6], F32, tag="bnstf", name="bnstf")
                for g in range(3):
                    nc.vector.bn_stats(bst[:, g, :],
                                       xf_nat[:, i, g * 256:(g + 1) * 256])
                nc.vector.bn_aggr(mvf[:, i, :], bst[:])
            rstdf = statp.tile([128, NT], F32, tag="rstdf", name="rstdf")
            emit_rsqrt(rstdf[:], mvf[:, :, 1], NT, "f")
            nmuf = statp.tile([128, NT], F32, tag="nmuf", name="nmuf")
            nc.scalar.mul(nmuf[:], mvf[:, :, 0], -1.0)
            st["xfnT"] = kvp.tile([128, NT, KTD, 128], BF16, tag="xfnT",
                                  name="xfnT")
            for i in range(NT):
                xfn_t = xfp.tile([128, TD], BF16, tag="xfn", name="xfn")
                nc.vector.tensor_scalar(xfn_t[:], xf_nat[:, i, :],
                                        nmuf[:, i:i + 1], rstdf[:, i:i + 1],
                                        op0=ALU.add, op1=ALU.mult)
                nc.sync.dma_start_transpose(st["xfnT"][:, i], xfn_t[:])

        def prep_xstats(st, i0, i1):
            if "mvx" not in st:
                st["mvx"] = statp.tile([128, TI, 2], F32, tag="mvx",
                                       name="mvx")
            for i in range(i0, i1):
                bst = statp.tile([128, 2, 6], F32, tag="bnstx", name="bnstx")
                for g in range(2):
                    nc.vector.bn_stats(bst[:, g, :],
                                       st["x_nat"][:, i, g * 512:(g + 1) * 512])
                nc.vector.bn_aggr(st["mvx"][:, i, :], bst[:])

        def prep_xfinish(st):
            st["rstdx"] = statp.tile([128, TI], F32, tag="rstdx", name="rstdx")
            emit_rsqrt(st["rstdx"][:], st["mvx"][:, :, 1], TI, "x")
            st["nmux"] = statp.tile([128, TI], F32, tag="nmux", name="nmux")
            nc.scalar.mul(st["nmux"][:], st["mvx"][:, :, 0], -1.0)

        def emit_kv(st):
            xfnT = st["xfnT"]
            exp_k = kvp.tile([128, NT, D], BF16, tag="expk", name="expk")
            v_sb = kvp.tile([128, NT, D], BF16, tag="vsb", name="vsb")
            for nt in range(NT):
                for ch in range(NCH):
                    cs = slice(ch * 512, (ch + 1) * 512)
                    pk = psq.tile([128, 512], F32, tag="ps", name="pk")
                    for kt in range(KTD):
                        nc.tensor.matmul(pk[:], xfnT[:, nt, kt, :],
                                         wk_sb[:, kt, cs],
                                         start=(kt == 0),
                                         stop=(kt == KTD - 1 and not qkv_bias))
                    if qkv_bias:
                        nc.tensor.matmul(pk[:], ones_r[:], bke_r[0:1, cs],
                                         start=False, stop=True)
                    nc.scalar.activation(exp_k[:, nt, cs], pk[:], AF.Exp)
                    pv = psq.tile([128, 512], F32, tag="ps", name="pv")
                    for kt in range(KTD):
                        nc.tensor.matmul(pv[:], xfnT[:, nt, kt, :],
                                         wv_sb[:, kt, cs],
                                         start=(kt == 0),
                                         stop=(kt == KTD - 1 and not qkv_bias))
                    if qkv_bias:
                        nc.tensor.matmul(pv[:], ones_r[:], bve_r[0:1, cs],
                                         start=False, stop=True)
                    nc.scalar.copy(v_sb[:, nt, cs], pv[:])

            pks = psa.tile([128, KD], F32, tag="pks", name="pks")
            for j in range(KD):
                for nt in range(NT):
                    nc.tensor.matmul(pks[:, j:j + 1],
                                     exp_k[:, nt, j * 128:(j + 1) * 128],
                                     ones_c[:], start=(nt == 0), stop=(nt == 1))
            r_k = statp.tile([128, KD], F32, tag="rk", name="rk")
            nc.vector.reciprocal(r_k[:], pks[:])

            patt = psa.tile([128, 512], F32, tag="patt", name="patt")
            for h in range(H):
                rp = slice((h % 2) * 64, (h % 2) * 64 + 64)
                cp = slice((h // 2) * 64, (h // 2) * 64 + 64)
                hs = slice(h * 64, (h + 1) * 64)
                for nt in range(NT):
                    nc.tensor.matmul(patt[rp, cp], exp_k[:, nt, hs],
                                     v_sb[:, nt, hs],
                                     start=(nt == 0), stop=(nt == 1))
            # block-diagonal per head pair: [0:64,0:64]=head 2j, [64:,64:]=2j+1
            attn_s = kvp.tile([128, KD, 128], BF16, tag="attns", name="attns")
            nc.vector.memset(attn_s[:], 0.0)
            for j in range(KD):
                nc.vector.tensor_scalar_mul(attn_s[0:64, j, 0:64],
                                            patt[0:64, j * 64:(j + 1) * 64],
                                            r_k[0:64, j:j + 1])
                nc.vector.tensor_scalar_mul(attn_s[64:128, j, 64:128],
                                            patt[64:128, j * 64:(j + 1) * 64],
                                            r_k[64:128, j:j + 1])
            st["attn_s"] = attn_s

        # ---- prefetch batch 0 (before the emb phase holds up the queue) ----
        prep = prep_loads(0)
        prep_xf(prep)
        prep_xstats(prep, 0, TI)
        prep_xfinish(prep)

        # ---- remaining weights ----
        wq_sb = wpool.tile([128, KD, D], BF16)
        nc.sync.dma_start(wq_sb[:], d_wq[:].rearrange("(i p) m -> p i m", p=128))
        wo_sb = wpool.tile([128, KD, D], BF16)
        nc.sync.dma_start(wo_sb[:], d_wo[:].rearrange("(i p) m -> p i m", p=128))

        # ---- emb / FiLM phase (all 4 batches at once) ----
        with tc.tile_pool(name="wemb", bufs=1) as wep, \
             tc.tile_pool(name="etmp", bufs=1) as ep, \
             tc.tile_pool(name="pse", bufs=2, space=bass.MemorySpace.PSUM) as pse:
            embb_r = ep.tile([1, D2], BF16)
            nc.sync.dma_start(embb_r[:], d_embb[None, :])
            fg_r = ep.tile([1, D], BF16)
            nc.sync.dma_start(fg_r[:], d_fg[None, :])
            fb_r = ep.tile([1, D], BF16)
            nc.sync.dma_start(fb_r[:], d_fb[None, :])
            emb_sb = ep.tile([BPC, TE], BF16)
            nc.sync.dma_start(emb_sb[:], d_emb[:])
            # silu(emb) = (emb/2)*(1+tanh(emb/2)) -- stays on the exp table set
            th = ep.tile([BPC, TE], BF16)
            nc.scalar.activation(th[:], emb_sb[:], AF.Tanh, scale=0.5)
            hemb = ep.tile([BPC, TE], BF16)
            nc.scalar.mul(hemb[:], emb_sb[:], 0.5)
            semb = ep.tile([BPC, TE], BF16)
            nc.vector.scalar_tensor_tensor(semb[:], th[:], 1.0, hemb[:],
                                           op0=ALU.add, op1=ALU.mult)
            embT = ep.tile([128, KTE, BPC], BF16)
            for c in range(KTE):
                pst = pse.tile([128, BPC], BF16, tag="pst")
                nc.tensor.transpose(pst[:], semb[:, c * 128:(c + 1) * 128],
                                    ident[0:BPC, 0:BPC])
                nc.vector.tensor_copy(embT[:, c, :], pst[:])
            e_sb = ep.tile([BPC, D2], BF16)
            for ch in range(D2 // 512):
                # wemb streamed in 512-col chunks (2MB each) -- the full
                # [TE, 2D] weight would not fit next to the batch pools
                wemb_c = wep.tile([128, KTE, 512], BF16, tag="wembc",
                                  name="wembc")
                nc.sync.dma_start(
                    wemb_c[:],
                    d_wemb[:, ch * 512:(ch + 1) * 512].rearrange(
                        "(i p) m -> p i m", p=128))
                pe = pse.tile([BPC, 512], F32, tag="pe")
                for kt in range(KTE):
                    nc.tensor.matmul(pe[:], embT[:, kt, :],
                                     wemb_c[:, kt, :],
                                     start=(kt == 0), stop=False)
                nc.tensor.matmul(pe[:], ones_r[0:1, 0:BPC],
                                 embb_r[0:1, ch * 512:(ch + 1) * 512],
                                 start=False, stop=True)
                nc.vector.tensor_copy(e_sb[:, ch * 512:(ch + 1) * 512], pe[:])
            # halved FiLM rows: A' = (fg/2)*(1+scale), B' = (fb/2)*(1+scale)
            #                   + shift/2   (fg/fb arrive pre-halved)
            fg4 = ep.tile([BPC, D], BF16)
            nc.gpsimd.partition_broadcast(fg4[:], fg_r[:], channels=BPC)
            fb4 = ep.tile([BPC, D], BF16)
            nc.gpsimd.partition_broadcast(fb4[:], fb_r[:], channels=BPC)
            tall = ep.tile([BPC, D], BF16)
            nc.vector.tensor_scalar_add(tall[:], e_sb[:, 0:D], 1.0)
            nc.vector.tensor_mul(a_bf[:], tall[:], fg4[:])
            btmp = ep.tile([BPC, D], BF16)
            nc.vector.tensor_mul(btmp[:], tall[:], fb4[:])
            nc.vector.scalar_tensor_tensor(b_bf[:], e_sb[:, D:D2], 0.5,
                                           btmp[:], op0=ALU.mult, op1=ALU.add)

        # ---- t-loop pools (allocated after emb scratch is released) ----
        xnp = ctx.enter_context(tc.tile_pool(name="xn", bufs=2))
        xntp = ctx.enter_context(tc.tile_pool(name="xnT", bufs=2))
        qp = ctx.enter_context(tc.tile_pool(name="q", bufs=2))
        qtp = ctx.enter_context(tc.tile_pool(name="qT", bufs=2))
        hp = ctx.enter_context(tc.tile_pool(name="h", bufs=2))
        htp = ctx.enter_context(tc.tile_pool(name="hT", bufs=3))
        outp = ctx.enter_context(tc.tile_pool(name="o", bufs=2))
        psq = ctx.enter_context(
            tc.tile_pool(name="psq", bufs=2, space=bass.MemorySpace.PSUM))
        psy = ctx.enter_context(
            tc.tile_pool(name="psy", bufs=2, space=bass.MemorySpace.PSUM))
        pso = ctx.enter_context(
            tc.tile_pool(name="pso", bufs=2, space=bass.MemorySpace.PSUM))
        psa = ctx.enter_context(
            tc.tile_pool(name="psa", bufs=1, space=bass.MemorySpace.PSUM))

        prep_ab(prep, 0)

        for b in range(BPC):
            emit_kv(prep)
            x_nat = prep["x_nat"]
            nmux, rstdx = prep["nmux"], prep["rstdx"]
            a_bc, b_bc = prep["a_bc"], prep["b_bc"]
            attn_s = prep["attn_s"]
            nxt = None

            # ---------- software-pipelined t-tile loop ----------
            xnT_tiles = {}
            qT_tiles = {}
            hT_tiles = {}

            # prologue: xnT(0)
            xn_t = xnp.tile([128, D], BF16, tag="xn", name="xn0")
            nc.vector.tensor_scalar(xn_t[:], x_nat[:, 0, :], nmux[:, 0:1],
                                    rstdx[:, 0:1], op0=ALU.add, op1=ALU.mult)
            xnT_tiles[0] = xntp.tile([128, KD, 128], BF16, tag="xnT",
                                     name="xnT0")
            nc.sync.dma_start_transpose(xnT_tiles[0][:], xn_t[:])

            for it in range(TI + 2):
                # ---- spread next-batch prep across the loop ----
                if b + 1 < BPC:
                    if it == 0:
                        nxt = prep_loads(b + 1)
                        prep_ab(nxt, b + 1)
                    elif it == 2:
                        prep_xf(nxt)
                    elif it == 3:
                        prep_xstats(nxt, 0, 4)
                    elif it == 4:
                        prep_xstats(nxt, 4, TI)
                    elif it == 5:
                        prep_xfinish(nxt)

                # ---- prep xn/xnT for tile it+1 ----
                if it + 1 < TI:
                    xn_t = xnp.tile([128, D], BF16, tag="xn", name="xnt")
                    nc.vector.tensor_scalar(
                        xn_t[:], x_nat[:, it + 1, :], nmux[:, it + 1:it + 2],
                        rstdx[:, it + 1:it + 2], op0=ALU.add, op1=ALU.mult)
                    xnT_tiles[it + 1] = xntp.tile([128, KD, 128], BF16,
                                                  tag="xnT", name="xnTt")
                    nc.sync.dma_start_transpose(xnT_tiles[it + 1][:], xn_t[:])

                # ---- stage A: Q-proj / exp / softmax / qT for tile it ----
                if it < TI:
                    xnT_t = xnT_tiles.pop(it)
                    exp_q = qp.tile([128, D], BF16, tag="expq", name="expq")
                    s_q = colp.tile([128, H], F32, tag="sq", name="sq")
                    for ch in range(NCH):
                        cs = slice(ch * 512, (ch + 1) * 512)
                        pq = psq.tile([128, 512], F32, tag="ps", name="pq")
                        for kt in range(KD):
                            nc.tensor.matmul(
                                pq[:], xnT_t[:, kt, :], wq_sb[:, kt, cs],
                                start=(kt == 0),
                                stop=(kt == KD - 1 and not qkv_bias))
                        if qkv_bias:
                            nc.tensor.matmul(pq[:], ones_r[:], bqe_r[0:1, cs],
                                             start=False, stop=True)
                        nc.scalar.activation(exp_q[:, cs], pq[:], AF.Exp)
                        nc.vector.reduce_sum(
                            s_q[:, ch * 8:(ch + 1) * 8],
                            exp_q[:, cs].rearrange("p (h c) -> p h c", c=C),
                            axis=mybir.AxisListType.X)
                    r_qf = colp.tile([128, H], F32, tag="rqf", name="rqf")
                    nc.vector.reciprocal(r_qf[:], s_q[:])
                    r_qb = colp.tile([128, H], BF16, tag="rqb", name="rqb")
                    nc.scalar.copy(r_qb[:], r_qf[:])
                    qsoft = qp.tile([128, D], BF16, tag="qsoft", name="qsoft")
                    rq_bc = bass.AP(tensor=r_qb.tensor, offset=r_qb.offset,
                                    ap=[[r_qb.ap[0][0], 128], [1, H], [0, C]])
                    nc.vector.tensor_mul(
                        qsoft[:].rearrange("p (h c) -> p h c", c=C),
                        exp_q[:].rearrange("p (h c) -> p h c", c=C), rq_bc)
                    qT_tiles[it] = qtp.tile([128, KD, 128], BF16, tag="qT",
                                            name="qTt")
                    nc.sync.dma_start_transpose(qT_tiles[it][:], qsoft[:])

                # ---- stage B: y / LN / FiLM / silu / hT for tile it-1 ----
                tj = it - 1
                if 0 <= tj < TI:
                    qT_t = qT_tiles.pop(tj)
                    pys = [psy.tile([128, 512], F32, tag="py", name=f"py{ch}")
                           for ch in range(NCH)]
                    for j in range(KD):
                        nc.tensor.matmul(
                            pys[j // 4][:, (j % 4) * 128:(j % 4) * 128 + 128],
                            qT_t[:, j, :], attn_s[:, j, :],
                            start=True, stop=True)
                    sty = colp.tile([128, 2, 6], F32, tag="bnsty", name="bnsty")
                    nc.vector.bn_stats(sty[:, 0, :], pys[0][:])
                    nc.vector.bn_stats(sty[:, 1, :], pys[1][:])
                    mvy = colp.tile([128, 2], F32, tag="mvy", name="mvy")
                    nc.vector.bn_aggr(mvy[:], sty[:])
                    rstdy = colp.tile([128, 1], F32, tag="rstdy", name="rstdy")
                    emit_rsqrt(rstdy[:], mvy[:, 1:2], 1, "y")
                    nmry = colp.tile([128, 1], F32, tag="nmry", name="nmry")
                    nc.vector.scalar_tensor_tensor(nmry[:], mvy[:, 0:1], -1.0,
                                                   rstdy[:], op0=ALU.mult,
                                                   op1=ALU.mult)
                    silu_h = hp.tile([128, D], BF16, tag="siluh", name="siluh")
                    for ch in range(NCH):
                        cs = slice(ch * 512, (ch + 1) * 512)
                        stdt = hp.tile([128, 512], BF16, tag="stdt",
                                       name="stdt")
                        nc.scalar.activation(stdt[:], pys[ch][:], AF.Identity,
                                             bias=nmry[:], scale=rstdy[:])
                        film = hp.tile([128, 512], BF16, tag="film",
                                       name="film")
                        nc.vector.tensor_mul(film[:], stdt[:], a_bc[:, cs])
                        nc.vector.tensor_add(film[:], film[:], b_bc[:, cs])
                        # film is h/2; silu(h) = film*(1+tanh(film))
                        tht = hp.tile([128, 512], BF16, tag="tht", name="tht")
                        nc.scalar.activation(tht[:], film[:], AF.Tanh)
                        nc.vector.scalar_tensor_tensor(
                            silu_h[:, cs], tht[:], 1.0, film[:],
                            op0=ALU.add, op1=ALU.mult)
                    hT_tiles[tj] = htp.tile([128, KD, 128], BF16, tag="hT",
                                            name="hTt")
                    nc.sync.dma_start_transpose(hT_tiles[tj][:], silu_h[:])

                # ---- stage C: out-proj + residual + store for tile it-2 ----
                tk = it - 2
                if tk >= 0:
                    hT_t = hT_tiles.pop(tk)
                    ob = outp.tile([128, D], BF16, tag="ob", name="ob")
                    for ch in range(NCH):
                        cs = slice(ch * 512, (ch + 1) * 512)
                        po = pso.tile([128, 512], F32, tag="po", name="po")
                        for j in range(KD):
                            nc.tensor.matmul(
                                po[:], hT_t[:, j, :], wo_sb[:, j, cs],
                                start=(j == 0),
                                stop=(j == KD - 1 and not out_bias))
                        if out_bias:
                            nc.tensor.matmul(po[:], ones_r[:],
                                             outb_r[0:1, cs],
                                             start=False, stop=True)
                        nc.scalar.copy(ob[:, cs], po[:])
                    o_sb = outp.tile([128, D], BF16, tag="osb", name="osb")
                    nc.vector.tensor_add(o_sb[:], ob[:], x_nat[:, tk, :])
                    nc.sync.dma_start(
                        d_out[b, tk * 128:(tk + 1) * 128, :], o_sb[:])

            if nxt is not None:
                prep = nxt

    nc.compile()
    return nc


def _get_program(qkv_bias, out_bias):
    key = (qkv_bias, out_bias)
    if key not in _PROGRAMS:
        _PROGRAMS[key] = _build_program(qkv_bias, out_bias)
    return _PROGRAMS[key]


def _prep_inputs(inputs):
    f = lambda k: np.asarray(inputs[k], np.float32)
    x, xf, emb = f("x"), f("xf"), f("emb")
    norm_g, norm_b = f("norm_g"), f("norm_b")
    tnorm_g, tnorm_b = f("tnorm_g"), f("tnorm_b")
    Wq, bq, Wk, bk, Wv, bv = f("Wq"), f("bq"), f("Wk"), f("bk"), f("Wv"), f("bv")
    emb_W, emb_b = f("emb_W"), f("emb_b")
    fg, fb = f("fnorm_g"), f("fnorm_b")
    out_W, out_b = f("out_W"), f("out_b")

    wq_e = norm_g[:, None] * Wq
    wk_e = tnorm_g[:, None] * Wk
    wv_e = tnorm_g[:, None] * Wv
    bqe = bq + norm_b @ Wq
    bke = bk + tnorm_b @ Wk
    bve = bv + tnorm_b @ Wv
    qkv_bias = bool(np.any(bqe) or np.any(bke) or np.any(bve))
    out_bias = bool(np.any(out_b))
    shared = {
        "wq": wq_e.astype(NBF), "wk": wk_e.astype(NBF), "wv": wv_e.astype(NBF),
        "wo": out_W.astype(NBF), "wemb": emb_W.astype(NBF),
        "bqe": bqe.astype(NBF), "bke": bke.astype(NBF), "bve": bve.astype(NBF),
        "outb": out_b.astype(NBF), "embb": emb_b.astype(NBF),
        "fg": (fg * 0.5).astype(NBF), "fb": (fb * 0.5).astype(NBF),
    }
    xbf = x.astype(NBF)
    xfbf = xf.astype(NBF)
    in_maps = []
    for i in range(NCORES):
        s = slice(i * BPC, (i + 1) * BPC)
        m = dict(shared)
        m["xbf"] = xbf[s]
        m["xfbf"] = xfbf[s]
        m["emb"] = emb[s].astype(NBF)
        in_maps.append(m)
    return in_maps, qkv_bias, out_bias


def run(inputs, trace=False):
    in_maps, qkv_bias, out_bias = _prep_inputs(inputs)
    nc = _get_program(qkv_bias, out_bias)
    res = run_bass_kernel_spmd(nc, in_maps, core_ids=list(range(NCORES)),
                               trace=trace)
    out = np.concatenate(
        [res.results[i]["out"].astype(np.float32) for i in range(NCORES)],
        axis=0)
    return out, res


def kernel(**inputs):
    out, _ = run(inputs, trace=False)
    return out


# revision 15
# speedup vs baseline: 2.5368x; 1.1266x over previous
"""Trainium2 Bass kernel for nn_CrossAttention (linear cross-attention block).

Computation (per batch b):
  xn  = LN(x[b]; norm_g, norm_b)                 [T, D]
  xfn = LN(xf[b]; tnorm_g, tnorm_b)              [N, TD]
  q   = softmax_c((xn @ Wq + bq).reshape(T,H,C))
  k   = softmax_n((xfn @ Wk + bk).reshape(N,H,C))
  v   = (xfn @ Wv + bv).reshape(N,H,C)
  attn= einsum('nhc,nhd->hcd', k, v); y = einsum('thc,hcd->thd', q, attn)
  e   = silu(emb) @ emb_W + emb_b; scale, shift = split(e)
  h   = LN(y; fnorm_g, fnorm_b) * (1+scale) + shift
  out = x + silu(h) @ out_W + out_b

Sharding: pure data-parallel over batch B=32 across 8 NeuronCores (4 each).

Device strategy:
  - LN gain folded into projection weights on the host. x / xf are
    normalized on-chip in natural layout (per-partition -mu / 1/std via one
    DVE tensor_scalar per tile), so projections are plain matmuls and the
    exp/copy PSUM drains need no per-row scale. Bias rank-1 folds are only
    emitted when biases are nonzero (they are zero for this model).
  - The scalar engine stays on ONE activation table set (exp_and_others:
    exp/tanh/identity/copy/square) for the whole kernel: 1/sqrt(var+eps)
    is computed on DVE with the bit-trick + 2 Newton steps, and silu(x) is
    x/2*(1+tanh(x/2)) with the 1/2 folded into the FiLM A/B coefficients
    (host passes fnorm_g/2, fnorm_b/2). This removes ~110 ACT_TABLE_LOADs
    (~1.5us each) that otherwise thrash between exp/sqrt/silu sets.
  - All transposes via the DMA xbar engine with multi-block destinations:
    one instruction per [128, D] tile ([128, KD, 128] dst), not one per
    128x128 block: flat ~1.2us sync-queue cost per instruction.
  - t-tile loop is software-pipelined with a 2-deep skew: per iteration the
    PE runs Q-proj(ti), y-matmul(ti-1), out-proj(ti-2) so the exp/softmax/
    transpose and LN/FiLM/silu/transpose tails of each tile hide under the
    next tiles' matmuls and the PE stays warm (HAM K=8/8). The next batch's
    x/xf loads + LN stats are spread across the current batch's t-loop.
  - Dual softmax: q-softmax over C is a grouped free-dim reduce + broadcast
    multiply; k-softmax over N folds into a per-partition reciprocal scale
    of the (exp_k^T v) head matmuls.
  - Residual uses the raw bf16 x kept resident in SBUF (ACT drains the out
    PSUM, DVE adds at bf16 2x rate); output stored bf16, upcast on host.
"""

from contextlib import ExitStack

import numpy as np
import ml_dtypes

import concourse.bass as bass
import concourse.mybir as mybir
import concourse.tile as tile
from concourse import bacc, library_config
from concourse.bass_utils import run_bass_kernel_spmd
from concourse.masks import make_identity

# problem shapes (hardcoded per contract)
B, T, N, D, TD, H, C, TE = 32, 1024, 256, 1024, 768, 16, 64, 2048
D2 = 2 * D
EPS = 1e-5
NCORES = 8
BPC = B // NCORES           # batches per core
TI = T // 128               # 8 t-tiles
KD = D // 128               # 8 k-tiles over D
KTD = TD // 128             # 6 k-tiles over TD
KTE = TE // 128             # 16 k-tiles over TE
NT = N // 128               # 2 n-tiles
NCH = D // 512              # 2 free 512-chunks over D

F32 = mybir.dt.float32
BF16 = mybir.dt.bfloat16
I32 = mybir.dt.int32
AF = mybir.ActivationFunctionType
ALU = mybir.AluOpType
NBF = ml_dtypes.bfloat16
MAGIC = 0x5F3759DF

_PROGRAMS = {}  # cached (nc) builds keyed by bias flags


def _build_program(qkv_bias, out_bias):
    nc = bacc.Bacc("TRN2", target_bir_lowering=False, debug=False,
                   num_devices=NCORES)

    # ---- DRAM I/O ----
    d_xbf = nc.dram_tensor("xbf", [BPC, T, D], BF16, kind="ExternalInput")
    d_xfbf = nc.dram_tensor("xfbf", [BPC, N, TD], BF16, kind="ExternalInput")
    d_emb = nc.dram_tensor("emb", [BPC, TE], BF16, kind="ExternalInput")
    d_wq = nc.dram_tensor("wq", [D, D], BF16, kind="ExternalInput")
    d_wk = nc.dram_tensor("wk", [TD, D], BF16, kind="ExternalInput")
    d_wv = nc.dram_tensor("wv", [TD, D], BF16, kind="ExternalInput")
    d_wo = nc.dram_tensor("wo", [D, D], BF16, kind="ExternalInput")
    d_wemb = nc.dram_tensor("wemb", [TE, D2], BF16, kind="ExternalInput")
    d_bqe = nc.dram_tensor("bqe", [D], BF16, kind="ExternalInput")
    d_bke = nc.dram_tensor("bke", [D], BF16, kind="ExternalInput")
    d_bve = nc.dram_tensor("bve", [D], BF16, kind="ExternalInput")
    d_outb = nc.dram_tensor("outb", [D], BF16, kind="ExternalInput")
    d_embb = nc.dram_tensor("embb", [D2], BF16, kind="ExternalInput")
    d_fg = nc.dram_tensor("fg", [D], BF16, kind="ExternalInput")   # fnorm_g/2
    d_fb = nc.dram_tensor("fb", [D], BF16, kind="ExternalInput")   # fnorm_b/2
    d_out = nc.dram_tensor("out", [BPC, T, D], BF16, kind="ExternalOutput")

    with tile.TileContext(nc) as tc, ExitStack() as ctx:
        wpool = ctx.enter_context(tc.tile_pool(name="weights", bufs=1))
        cpool = ctx.enter_context(tc.tile_pool(name="consts", bufs=1))

        # ---- persistent weights (K/V first so batch-0 KV starts early) ----
        wk_sb = wpool.tile([128, KTD, D], BF16)
        nc.sync.dma_start(wk_sb[:], d_wk[:].rearrange("(i p) m -> p i m", p=128))
        wv_sb = wpool.tile([128, KTD, D], BF16)
        nc.sync.dma_start(wv_sb[:], d_wv[:].rearrange("(i p) m -> p i m", p=128))

        # ---- constants ----
        def row_const(dram, n, tag):
            t = cpool.tile([1, n], BF16, tag=tag)
            nc.sync.dma_start(t[:], dram[None, :])
            return t
        bqe_r = row_const(d_bqe, D, "bqe_r") if qkv_bias else None
        bke_r = row_const(d_bke, D, "bke_r") if qkv_bias else None
        bve_r = row_const(d_bve, D, "bve_r") if qkv_bias else None
        outb_r = row_const(d_outb, D, "outb_r") if out_bias else None
        ones_r = cpool.tile([1, 128], BF16)
        nc.vector.memset(ones_r[:], 1.0)
        ones_c = cpool.tile([128, 1], BF16)
        nc.vector.memset(ones_c[:], 1.0)
        ones2 = cpool.tile([128, 2], BF16)
        nc.vector.memset(ones2[:], 0.0)
        nc.vector.memset(ones2[0:64, 0:1], 1.0)
        nc.vector.memset(ones2[64:128, 1:2], 1.0)
        magic_c = cpool.tile([128, TI], I32)
        nc.vector.memset(magic_c[:], MAGIC)
        ident = cpool.tile([128, 128], BF16)
        make_identity(nc, ident[:])
        nc.gpsimd.load_library(library_config.attnmlp)

        a_bf = cpool.tile([BPC, D], BF16)   # FiLM A/2 rows (one per batch)
        b_bf = cpool.tile([BPC, D], BF16)   # FiLM B/2 rows

        # ---- batch-phase pools (prep + KV only; t-loop pools open after
        # the emb phase releases its scratch) ----
        xp = ctx.enter_context(tc.tile_pool(name="x", bufs=2))
        xfp = ctx.enter_context(tc.tile_pool(name="xf", bufs=2))
        statp = ctx.enter_context(tc.tile_pool(name="stat", bufs=2))
        kvp = ctx.enter_context(tc.tile_pool(name="kv", bufs=2))
        abp = ctx.enter_context(tc.tile_pool(name="ab", bufs=2))
        colp = ctx.enter_context(tc.tile_pool(name="cols", bufs=2))

        def emit_rsqrt(out, var_src, w, tag, iters=2):
            """out[128, w] f32 = 1/sqrt(var_src + EPS).

            DVE-only (bit-trick + Newton steps) so the ACT engine never
            has to swap to the sqrt table set. 1 iter: ~0.2% rel err."""
            ve = colp.tile([128, w], F32, tag=tag + "ve", name=tag + "ve")
            nc.vector.tensor_scalar_add(ve[:], var_src, EPS)
            t1 = colp.tile([128, w], I32, tag=tag + "t1", name=tag + "t1")
            nc.vector.tensor_scalar(t1[:], ve[:].bitcast(I32), 1, None,
                                    op0=ALU.arith_shift_right)
            nc.vector.scalar_tensor_tensor(out.bitcast(I32), magic_c[:, :w],
                                           0, t1[:], op0=ALU.add,
                                           op1=ALU.subtract)
            aa = colp.tile([128, w], F32, tag=tag + "aa", name=tag + "aa")
            cc = colp.tile([128, w], F32, tag=tag + "cc", name=tag + "cc")
            for _ in range(iters):
                nc.vector.tensor_mul(aa[:], ve[:], out)
                nc.vector.tensor_mul(aa[:], aa[:], out)
                nc.vector.tensor_scalar(cc[:], aa[:], -0.5, 1.5,
                                        op0=ALU.mult, op1=ALU.add)
                nc.vector.tensor_mul(out, out, cc[:])

        # ---------- per-batch prep, split into chunks for pipelining ------
        def prep_ab(st, b):
            # must be emitted after the emb phase has written a_bf/b_bf --
            # Tile dependencies only flow forward in trace order
            arow = abp.tile([1, D], BF16, tag="arow", name="arow")
            nc.sync.dma_start(arow[:], a_bf[b:b + 1, :])
            brow = abp.tile([1, D], BF16, tag="brow", name="brow")
            nc.sync.dma_start(brow[:], b_bf[b:b + 1, :])
            st["a_bc"] = abp.tile([128, D], BF16, tag="abc", name="abc")
            nc.gpsimd.partition_broadcast(st["a_bc"][:], arow[:], channels=128)
            st["b_bc"] = abp.tile([128, D], BF16, tag="bbc", name="bbc")
            nc.gpsimd.partition_broadcast(st["b_bc"][:], brow[:], channels=128)

        def prep_loads(b):
            st = {}
            st["xf_nat"] = xfp.tile([128, NT, TD], BF16, tag="xfnat",
                                    name="xfnat")
            nc.sync.dma_start(st["xf_nat"][:],
                              d_xfbf[b].rearrange("(i p) m -> p i m", p=128))
            st["x_nat"] = xp.tile([128, TI, D], BF16, tag="xnat", name="xnat")
            nc.sync.dma_start(st["x_nat"][:],
                              d_xbf[b].rearrange("(i p) m -> p i m", p=128))
            return st

        def prep_xf(st):
            xf_nat = st["xf_nat"]
            mvf = statp.tile([128, NT, 2], F32, tag="mvf", name="mvf")
            for i in range(NT):
                bst = statp.tile([128, 3, 6], F32, tag="bnstf", name="bnstf")
                for g in range(3):
                    nc.vector.bn_stats(bst[:, g, :],
                                       xf_nat[:, i, g * 256:(g + 1) * 256])
                nc.vector.bn_aggr(mvf[:, i, :], bst[:])
            rstdf = statp.tile([128, NT], F32, tag="rstdf", name="rstdf")
            emit_rsqrt(rstdf[:], mvf[:, :, 1], NT, "f")
            bxf = statp.tile([128, NT], F32, tag="bxf", name="bxf")
            nc.vector.scalar_tensor_tensor(bxf[:], mvf[:, :, 0], -1.0,
                                           rstdf[:], op0=ALU.mult,
                                           op1=ALU.mult)
            st["xfnT"] = kvp.tile([128, NT, KTD, 128], BF16, tag="xfnT",
                                  name="xfnT")
            for i in range(NT):
                xfn_t = xfp.tile([128, TD], BF16, tag="xfn", name="xfn")
                nc.scalar.activation(xfn_t[:], xf_nat[:, i, :], AF.Identity,
                                     bias=bxf[:, i:i + 1],
                                     scale=rstdf[:, i:i + 1])
                nc.sync.dma_start_transpose(st["xfnT"][:, i], xfn_t[:])

        def prep_xstats(st, i0, i1):
            if "mvx" not in st:
                st["mvx"] = statp.tile([128, TI, 2], F32, tag="mvx",
                                       name="mvx")
            for i in range(i0, i1):
                bst = statp.tile([128, 2, 6], F32, tag="bnstx", name="bnstx")
                for g in range(2):
                    nc.vector.bn_stats(bst[:, g, :],
                                       st["x_nat"][:, i, g * 512:(g + 1) * 512])
                nc.vector.bn_aggr(st["mvx"][:, i, :], bst[:])

        def prep_xfinish(st):
            st["rstdx"] = statp.tile([128, TI], F32, tag="rstdx", name="rstdx")
            emit_rsqrt(st["rstdx"][:], st["mvx"][:, :, 1], TI, "x")
            st["bx"] = statp.tile([128, TI], F32, tag="bx", name="bx")
            nc.vector.scalar_tensor_tensor(st["bx"][:], st["mvx"][:, :, 0],
                                           -1.0, st["rstdx"][:],
                                           op0=ALU.mult, op1=ALU.mult)

        def emit_xnT(st, i):
            xn_t = xnp.tile([128, D], BF16, tag="xn", name="xn")
            nc.scalar.activation(xn_t[:], st["x_nat"][:, i, :], AF.Identity,
                                 bias=st["bx"][:, i:i + 1],
                                 scale=st["rstdx"][:, i:i + 1])
            t = xntp.tile([128, KD, 128], BF16, tag="xnT", name="xnT")
            nc.sync.dma_start_transpose(t[:], xn_t[:])
            return t

        def emit_kv(st):
            xfnT = st["xfnT"]
            exp_k = kvp.tile([128, NT, D], BF16, tag="expk", name="expk")
            v_sb = kvp.tile([128, NT, D], BF16, tag="vsb", name="vsb")
            for nt in range(NT):
                for ch in range(NCH):
                    cs = slice(ch * 512, (ch + 1) * 512)
                    pk = psq.tile([128, 512], F32, tag="ps", name="pk")
                    for kt in range(KTD):
                        nc.tensor.matmul(pk[:], xfnT[:, nt, kt, :],
                                         wk_sb[:, kt, cs],
                                         start=(kt == 0),
                                         stop=(kt == KTD - 1 and not qkv_bias))
                    if qkv_bias:
                        nc.tensor.matmul(pk[:], ones_r[:], bke_r[0:1, cs],
                                         start=False, stop=True)
                    nc.scalar.activation(exp_k[:, nt, cs], pk[:], AF.Exp)
                    pv = psq.tile([128, 512], F32, tag="ps", name="pv")
                    for kt in range(KTD):
                        nc.tensor.matmul(pv[:], xfnT[:, nt, kt, :],
                                         wv_sb[:, kt, cs],
                                         start=(kt == 0),
                                         stop=(kt == KTD - 1 and not qkv_bias))
                    if qkv_bias:
                        nc.tensor.matmul(pv[:], ones_r[:], bve_r[0:1, cs],
                                         start=False, stop=True)
                    nc.scalar.copy(v_sb[:, nt, cs], pv[:])

            pks = psa.tile([128, KD], F32, tag="kvps", name="pks")
            for j in range(KD):
                for nt in range(NT):
                    nc.tensor.matmul(pks[:, j:j + 1],
                                     exp_k[:, nt, j * 128:(j + 1) * 128],
                                     ones_c[:], start=(nt == 0), stop=(nt == 1))
            r_k = statp.tile([128, KD], F32, tag="rk", name="rk")
            nc.vector.reciprocal(r_k[:], pks[:])

            patt = psa.tile([128, 512], F32, tag="kvps", name="patt")
            for h in range(H):
                rp = slice((h % 2) * 64, (h % 2) * 64 + 64)
                cp = slice((h // 2) * 64, (h // 2) * 64 + 64)
                hs = slice(h * 64, (h + 1) * 64)
                for nt in range(NT):
                    nc.tensor.matmul(patt[rp, cp], exp_k[:, nt, hs],
                                     v_sb[:, nt, hs],
                                     start=(nt == 0), stop=(nt == 1))
            # block-diagonal per head pair: [0:64,0:64]=head 2j, [64:,64:]=2j+1
            attn_s = kvp.tile([128, KD, 128], BF16, tag="attns", name="attns")
            nc.vector.memset(attn_s[:], 0.0)
            for j in range(KD):
                nc.vector.tensor_scalar_mul(attn_s[0:64, j, 0:64],
                                            patt[0:64, j * 64:(j + 1) * 64],
                                            r_k[0:64, j:j + 1])
                nc.vector.tensor_scalar_mul(attn_s[64:128, j, 64:128],
                                            patt[64:128, j * 64:(j + 1) * 64],
                                            r_k[64:128, j:j + 1])
            st["attn_s"] = attn_s

        # ---- prefetch batch 0 (before the emb phase holds up the queue) ----
        psq = ctx.enter_context(
            tc.tile_pool(name="psq", bufs=2, space=bass.MemorySpace.PSUM))
        psa = ctx.enter_context(
            tc.tile_pool(name="psa", bufs=1, space=bass.MemorySpace.PSUM))

        prep = prep_loads(0)
        prep_xf(prep)
        prep_xstats(prep, 0, TI)
        prep_xfinish(prep)
        emit_kv(prep)   # batch-0 K/V runs while emb weights stream in

        # ---- remaining weights ----
        wq_sb = wpool.tile([128, KD, D], BF16)
        nc.sync.dma_start(wq_sb[:], d_wq[:].rearrange("(i p) m -> p i m", p=128))
        wo_sb = wpool.tile([128, KD, D], BF16)
        nc.sync.dma_start(wo_sb[:], d_wo[:].rearrange("(i p) m -> p i m", p=128))

        # ---- emb / FiLM phase (all 4 batches at once) ----
        with tc.tile_pool(name="wemb", bufs=1) as wep, \
             tc.tile_pool(name="etmp", bufs=1) as ep, \
             tc.tile_pool(name="pse", bufs=2, space=bass.MemorySpace.PSUM) as pse:
            embb_r = ep.tile([1, D2], BF16)
            nc.sync.dma_start(embb_r[:], d_embb[None, :])
            fg_r = ep.tile([1, D], BF16)
            nc.sync.dma_start(fg_r[:], d_fg[None, :])
            fb_r = ep.tile([1, D], BF16)
            nc.sync.dma_start(fb_r[:], d_fb[None, :])
            emb_sb = ep.tile([BPC, TE], BF16)
            nc.sync.dma_start(emb_sb[:], d_emb[:])
            # silu(emb) = (emb/2)*(1+tanh(emb/2)) -- stays on the exp table set
            th = ep.tile([BPC, TE], BF16)
            nc.scalar.activation(th[:], emb_sb[:], AF.Tanh, scale=0.5)
            hemb = ep.tile([BPC, TE], BF16)
            nc.scalar.mul(hemb[:], emb_sb[:], 0.5)
            semb = ep.tile([BPC, TE], BF16)
            nc.vector.scalar_tensor_tensor(semb[:], th[:], 1.0, hemb[:],
                                           op0=ALU.add, op1=ALU.mult)
            embT = ep.tile([128, KTE, BPC], BF16)
            for c in range(KTE):
                pst = pse.tile([128, BPC], BF16, tag="pst")
                nc.tensor.transpose(pst[:], semb[:, c * 128:(c + 1) * 128],
                                    ident[0:BPC, 0:BPC])
                nc.vector.tensor_copy(embT[:, c, :], pst[:])
            e_sb = ep.tile([BPC, D2], BF16)
            for ch in range(D2 // 512):
                # wemb streamed in 512-col chunks (2MB each) -- the full
                # [TE, 2D] weight would not fit next to the batch pools
                wemb_c = wep.tile([128, KTE, 512], BF16, tag="wembc",
                                  name="wembc")
                nc.sync.dma_start(
                    wemb_c[:],
                    d_wemb[:, ch * 512:(ch + 1) * 512].rearrange(
                        "(i p) m -> p i m", p=128))
                pe = pse.tile([BPC, 512], F32, tag="pe")
                for kt in range(KTE):
                    nc.tensor.matmul(pe[:], embT[:, kt, :],
                                     wemb_c[:, kt, :],
                                     start=(kt == 0), stop=False)
                nc.tensor.matmul(pe[:], ones_r[0:1, 0:BPC],
                                 embb_r[0:1, ch * 512:(ch + 1) * 512],
                                 start=False, stop=True)
                nc.vector.tensor_copy(e_sb[:, ch * 512:(ch + 1) * 512], pe[:])
            # halved FiLM rows: A' = (fg/2)*(1+scale), B' = (fb/2)*(1+scale)
            #                   + shift/2   (fg/fb arrive pre-halved)
            fg4 = ep.tile([BPC, D], BF16)
            nc.gpsimd.partition_broadcast(fg4[:], fg_r[:], channels=BPC)
            fb4 = ep.tile([BPC, D], BF16)
            nc.gpsimd.partition_broadcast(fb4[:], fb_r[:], channels=BPC)
            tall = ep.tile([BPC, D], BF16)
            nc.vector.tensor_scalar_add(tall[:], e_sb[:, 0:D], 1.0)
            nc.vector.tensor_mul(a_bf[:], tall[:], fg4[:])
            btmp = ep.tile([BPC, D], BF16)
            nc.vector.tensor_mul(btmp[:], tall[:], fb4[:])
            nc.vector.scalar_tensor_tensor(b_bf[:], e_sb[:, D:D2], 0.5,
                                           btmp[:], op0=ALU.mult, op1=ALU.add)

        # ---- t-loop pools (allocated after emb scratch is released) ----
        xnp = ctx.enter_context(tc.tile_pool(name="xn", bufs=2))
        xntp = ctx.enter_context(tc.tile_pool(name="xnT", bufs=2))
        qp = ctx.enter_context(tc.tile_pool(name="q", bufs=2))
        qtp = ctx.enter_context(tc.tile_pool(name="qT", bufs=2))
        hp = ctx.enter_context(tc.tile_pool(name="h", bufs=2))
        htp = ctx.enter_context(tc.tile_pool(name="hT", bufs=3))
        outp = ctx.enter_context(tc.tile_pool(name="o", bufs=2))
        psy = ctx.enter_context(
            tc.tile_pool(name="psy", bufs=2, space=bass.MemorySpace.PSUM))
        pso = ctx.enter_context(
            tc.tile_pool(name="pso", bufs=2, space=bass.MemorySpace.PSUM))

        prep_ab(prep, 0)

        for b in range(BPC):
            x_nat = prep["x_nat"]
            a_bc, b_bc = prep["a_bc"], prep["b_bc"]
            attn_s = prep["attn_s"]
            nxt = None

            # ---------- software-pipelined t-tile loop ----------
            qT_tiles = {}
            hT_tiles = {}
            xnT_tiles = prep.pop("xnT_tiles", None)
            if xnT_tiles is None:
                xnT_tiles = {0: emit_xnT(prep, 0)}

            for it in range(TI + 2):
                # ---- spread next-batch prep across the loop ----
                if b + 1 < BPC:
                    if it == 0:
                        nxt = prep_loads(b + 1)
                        prep_ab(nxt, b + 1)
                    elif it == 2:
                        prep_xf(nxt)
                    elif it == 3:
                        prep_xstats(nxt, 0, 4)
                    elif it == 4:
                        prep_xstats(nxt, 4, TI)
                    elif it == 5:
                        prep_xfinish(nxt)
                    elif it == 6:
                        # next batch's K/V fills the batch-boundary PE gap
                        emit_kv(nxt)
                    elif it == 7:
                        nxt["xnT_tiles"] = {0: emit_xnT(nxt, 0)}

                # ---- prep xn/xnT for tile it+1 ----
                if it + 1 < TI:
                    xnT_tiles[it + 1] = emit_xnT(prep, it + 1)

                # ---- stage A: Q-proj / exp / qT for tile it ----
                # (softmax normalization is deferred past the y-matmul)
                if it < TI:
                    xnT_t = xnT_tiles.pop(it)
                    exp_q = qp.tile([128, D], BF16, tag="expq", name="expq")
                    for ch in range(NCH):
                        cs = slice(ch * 512, (ch + 1) * 512)
                        pq = psq.tile([128, 512], F32, tag="ps", name="pq")
                        for kt in range(KD):
                            nc.tensor.matmul(
                                pq[:], xnT_t[:, kt, :], wq_sb[:, kt, cs],
                                start=(kt == 0),
                                stop=(kt == KD - 1 and not qkv_bias))
                        if qkv_bias:
                            nc.tensor.matmul(pq[:], ones_r[:], bqe_r[0:1, cs],
                                             start=False, stop=True)
                        nc.scalar.activation(exp_q[:, cs], pq[:], AF.Exp)
                    qT_tiles[it] = qtp.tile([128, KD, 128], BF16, tag="qT",
                                            name="qTt")
                    nc.sync.dma_start_transpose(qT_tiles[it][:], exp_q[:])

                # ---- stage B: y / LN / FiLM / silu / hT for tile it-1 ----
                tj = it - 1
                if 0 <= tj < TI:
                    qT_t = qT_tiles.pop(tj)
                    # per-(t, head) softmax sums via tiny masked matmuls
                    psS = psa.tile([128, H], F32, tag="psS", name="psS")
                    for j in range(KD):
                        nc.tensor.matmul(psS[:, 2 * j:2 * j + 2],
                                         qT_t[:, j, :], ones2[:],
                                         start=True, stop=True)
                    pys = [psy.tile([128, 512], F32, tag="py", name=f"py{ch}")
                           for ch in range(NCH)]
                    for j in range(KD):
                        nc.tensor.matmul(
                            pys[j // 4][:, (j % 4) * 128:(j % 4) * 128 + 128],
                            qT_t[:, j, :], attn_s[:, j, :],
                            start=True, stop=True)
                    r_q = colp.tile([128, H], F32, tag="rq", name="rq")
                    nc.vector.reciprocal(r_q[:], psS[:])
                    # y / S (grouped broadcast over each head's 64 channels)
                    ynorm = hp.tile([128, D], BF16, tag="ynorm", name="ynorm")
                    for ch in range(NCH):
                        cs = slice(ch * 512, (ch + 1) * 512)
                        sl = r_q[:, ch * 8:(ch + 1) * 8]
                        rq_bc = bass.AP(tensor=sl.tensor, offset=sl.offset,
                                        ap=[[sl.ap[0][0], 128], [1, 8],
                                            [0, C]])
                        nc.vector.tensor_mul(
                            ynorm[:, cs].rearrange("p (h c) -> p h c", c=C),
                            pys[ch][:].rearrange("p (h c) -> p h c", c=C),
                            rq_bc)
                    sty = colp.tile([128, 2, 6], F32, tag="bnsty", name="bnsty")
                    nc.vector.bn_stats(sty[:, 0, :], ynorm[:, 0:512])
                    nc.vector.bn_stats(sty[:, 1, :], ynorm[:, 512:1024])
                    mvy = colp.tile([128, 2], F32, tag="mvy", name="mvy")
                    nc.vector.bn_aggr(mvy[:], sty[:])
                    rstdy = colp.tile([128, 1], F32, tag="rstdy", name="rstdy")
                    emit_rsqrt(rstdy[:], mvy[:, 1:2], 1, "y", iters=1)
                    nmry = colp.tile([128, 1], F32, tag="nmry", name="nmry")
                    nc.vector.scalar_tensor_tensor(nmry[:], mvy[:, 0:1], -1.0,
                                                   rstdy[:], op0=ALU.mult,
                                                   op1=ALU.mult)
                    silu_h = hp.tile([128, D], BF16, tag="siluh", name="siluh")
                    for ch in range(NCH):
                        cs = slice(ch * 512, (ch + 1) * 512)
                        stdt = hp.tile([128, 512], BF16, tag="stdt",
                                       name="stdt")
                        nc.scalar.activation(stdt[:], ynorm[:, cs],
                                             AF.Identity,
                                             bias=nmry[:], scale=rstdy[:])
                        film = hp.tile([128, 512], BF16, tag="film",
                                       name="film")
                        nc.vector.tensor_mul(film[:], stdt[:], a_bc[:, cs])
                        nc.vector.tensor_add(film[:], film[:], b_bc[:, cs])
                        # film is h/2; silu(h) = film*(1+tanh(film))
                        tht = hp.tile([128, 512], BF16, tag="tht", name="tht")
                        nc.scalar.activation(tht[:], film[:], AF.Tanh)
                        nc.vector.scalar_tensor_tensor(
                            silu_h[:, cs], tht[:], 1.0, film[:],
                            op0=ALU.add, op1=ALU.mult)
                    hT_tiles[tj] = htp.tile([128, KD, 128], BF16, tag="hT",
                                            name="hTt")
                    nc.sync.dma_start_transpose(hT_tiles[tj][:], silu_h[:])

                # ---- stage C: out-proj + residual + store for tile it-2 ----
                tk = it - 2
                if tk >= 0:
                    hT_t = hT_tiles.pop(tk)
                    ob = outp.tile([128, D], BF16, tag="ob", name="ob")
                    for ch in range(NCH):
                        cs = slice(ch * 512, (ch + 1) * 512)
                        po = pso.tile([128, 512], F32, tag="po", name="po")
                        for j in range(KD):
                            nc.tensor.matmul(
                                po[:], hT_t[:, j, :], wo_sb[:, j, cs],
                                start=(j == 0),
                                stop=(j == KD - 1 and not out_bias))
                        if out_bias:
                            nc.tensor.matmul(po[:], ones_r[:],
                                             outb_r[0:1, cs],
                                             start=False, stop=True)
                        nc.scalar.copy(ob[:, cs], po[:])
                    o_sb = outp.tile([128, D], BF16, tag="osb", name="osb")
                    nc.vector.tensor_add(o_sb[:], ob[:], x_nat[:, tk, :])
                    nc.sync.dma_start(
                        d_out[b, tk * 128:(tk + 1) * 128, :], o_sb[:])

            if nxt is not None:
                prep = nxt

    nc.compile()
    return nc


def _get_program(qkv_bias, out_bias):
    key = (qkv_bias, out_bias)
    if key not in _PROGRAMS:
        _PROGRAMS[key] = _build_program(qkv_bias, out_bias)
    return _PROGRAMS[key]


def _prep_inputs(inputs):
    f = lambda k: np.asarray(inputs[k], np.float32)
    x, xf, emb = f("x"), f("xf"), f("emb")
    norm_g, norm_b = f("norm_g"), f("norm_b")
    tnorm_g, tnorm_b = f("tnorm_g"), f("tnorm_b")
    Wq, bq, Wk, bk, Wv, bv = f("Wq"), f("bq"), f("Wk"), f("bk"), f("Wv"), f("bv")
    emb_W, emb_b = f("emb_W"), f("emb_b")
    fg, fb = f("fnorm_g"), f("fnorm_b")
    out_W, out_b = f("out_W"), f("out_b")

    wq_e = norm_g[:, None] * Wq
    wk_e = tnorm_g[:, None] * Wk
    wv_e = tnorm_g[:, None] * Wv
    bqe = bq + norm_b @ Wq
    bke = bk + tnorm_b @ Wk
    bve = bv + tnorm_b @ Wv
    qkv_bias = bool(np.any(bqe) or np.any(bke) or np.any(bve))
    out_bias = bool(np.any(out_b))
    shared = {
        "wq": wq_e.astype(NBF), "wk": wk_e.astype(NBF), "wv": wv_e.astype(NBF),
        "wo": out_W.astype(NBF), "wemb": emb_W.astype(NBF),
        "bqe": bqe.astype(NBF), "bke": bke.astype(NBF), "bve": bve.astype(NBF),
        "outb": out_b.astype(NBF), "embb": emb_b.astype(NBF),
        "fg": (fg * 0.5).astype(NBF), "fb": (fb * 0.5).astype(NBF),
    }
    xbf = x.astype(NBF)
    xfbf = xf.astype(NBF)
    in_maps = []
    for i in range(NCORES):
        s = slice(i * BPC, (i + 1) * BPC)
        m = dict(shared)
        m["xbf"] = xbf[s]
        m["xfbf"] = xfbf[s]
        m["emb"] = emb[s].astype(NBF)
        in_maps.append(m)
    return in_maps, qkv_bias, out_bias


def run(inputs, trace=False):
    in_maps, qkv_bias, out_bias = _prep_inputs(inputs)
    nc = _get_program(qkv_bias, out_bias)
    res = run_bass_kernel_spmd(nc, in_maps, core_ids=list(range(NCORES)),
                               trace=trace)
    out = np.concatenate(
        [res.results[i]["out"].astype(np.float32) for i in range(NCORES)],
        axis=0)
    return out, res


def kernel(**inputs):
    out, _ = run(inputs, trace=False)
    return out
